# revision 18
# baseline (speedup 1.0000x reference)
"""Trainium2 Bass kernel for nn_DiTBlock (HGRN-attention DiT block).

Sharding: 8 cores = 4 batches x 2 half-sequences (1024 tokens each).
All big matmuls run as EXACT integer arithmetic on the fp8 PE path:
activations are int8-grid quantized (matching the reference bitlinear),
then split x = hi + lo with hi = 16*round(x/16) (multiples of 16, fp8-
exact) and lo = x - hi (|lo| <= 8, fp8-exact). A DoubleRow fp8 matmul
contracts the (hi, lo) pair against a stride-0-replicated ternary fp8
weight pair in one instruction -> 2x the bf16 matmul rate with
bit-identical results. The down-projection uses bf16 activations x fp8
weights (plain matmul) to keep the 8M-element h2 quant cheap.
The time-recurrence h_t = f_t*h_{t-1} + i_t uses the DVE
tensor_tensor_scan; the half-sequence boundary carry crosses cores via
AllGather + one-hot mask matmul. adaln params are computed on-device,
sharded 8 ways over the 6144 outputs and AllGathered.
"""
import functools
import numpy as np
import ml_dtypes

import concourse.bass as bass
import concourse.bacc as bacc_mod
import concourse.mybir as mybir
import concourse.tile as tile
from concourse.masks import make_identity
from concourse.bass_utils import run_bass_kernel_spmd

E4M3 = ml_dtypes.float8_e4m3
F32 = mybir.dt.float32
BF = mybir.dt.bfloat16
F8 = mybir.dt.float8e4
U32 = mybir.dt.uint32
AL = mybir.AluOpType
AF = mybir.ActivationFunctionType
AX = mybir.AxisListType
PM = mybir.MatmulPerfMode

B, T, D = 4, 2048, 1024
TOK = 1024          # tokens per core
NH, HD = 16, 64
MLP = 4096
N_CORES = 8
C_MAGIC = float(1.5 * 2 ** 23)
C16 = 16.0 * C_MAGIC
MAGIC_U32 = 0x5F3759DF


def _quant_w(w):
    invws = float(np.clip(np.abs(w).mean(dtype=np.float64), 1e-5, None))
    m = np.clip(np.round(w.astype(np.float64) / invws), -1, 1).astype(np.float32)
    return np.ascontiguousarray(m.astype(E4M3)), np.float32(invws)


def _rsqrt(nc, sb, x_ap, scale, bias, shape, tag):
    """out = rsqrt(x*scale + bias), Newton on DVE. Returns a new tile."""
    t = sb.tile(shape, F32, tag=tag + "_t", name=tag + "_t")
    nc.vector.tensor_scalar(out=t, in0=x_ap, scalar1=float(scale),
                            scalar2=float(bias), op0=AL.mult, op1=AL.add)
    y = sb.tile(shape, F32, tag=tag + "_y", name=tag + "_y")
    sh = sb.tile(shape, F32, tag=tag + "_s", name=tag + "_s")
    nc.vector.tensor_scalar(out=sh[:].bitcast(U32), in0=t[:].bitcast(U32),
                            scalar1=1, scalar2=None, op0=AL.logical_shift_right)
    mg = sb.tile(shape, F32, tag=tag + "_m", name=tag + "_m")
    nc.vector.memset(mg[:].bitcast(U32), MAGIC_U32)
    nc.vector.tensor_tensor(out=y[:].bitcast(U32), in0=mg[:].bitcast(U32),
                            in1=sh[:].bitcast(U32), op=AL.subtract)
    e = sb.tile(shape, F32, tag=tag + "_e", name=tag + "_e")
    for _ in range(3):
        nc.vector.tensor_tensor(out=e, in0=y, in1=y, op=AL.mult)
        nc.vector.tensor_tensor(out=e, in0=e, in1=t, op=AL.mult)
        nc.vector.tensor_scalar(out=e, in0=e, scalar1=-0.5, scalar2=1.5,
                                op0=AL.mult, op1=AL.add)
        nc.vector.tensor_tensor(out=y, in0=y, in1=e, op=AL.mult)
    return y


def _wpair(ap2):
    """[128, X] weight AP -> [128, 2, X] with stride-0 pair dim."""
    return bass.AP(tensor=ap2.tensor, offset=ap2.offset,
                   ap=[ap2.ap[0], [0, 2], ap2.ap[-1]])


def _build(iw):
    nc = bacc_mod.Bacc("TRN2", target_bir_lowering=False)

    x_sl = nc.declare_dram_parameter("x_sl", [TOK, D], F32, isOutput=False)
    c_cols = nc.declare_dram_parameter("c_cols", [128, 8, B], F32, isOutput=False)
    adw_sl = nc.declare_dram_parameter("adw_sl", [D, 768], F32, isOutput=False)
    adb_row = nc.declare_dram_parameter("adb_row", [1, 6 * D], F32, isOutput=False)
    mask8 = nc.declare_dram_parameter("mask8", [N_CORES, 1], F32, isOutput=False)
    bmask = nc.declare_dram_parameter("bmask", [B, 1], F32, isOutput=False)
    gnr = nc.declare_dram_parameter("gnr", [1, D], F32, isOutput=False)
    wiT = nc.declare_dram_parameter("wiT", [D, D], F8, isOutput=False)
    wfT = nc.declare_dram_parameter("wfT", [D, D], F8, isOutput=False)
    wgT = nc.declare_dram_parameter("wgT", [D, D], F8, isOutput=False)
    woT = nc.declare_dram_parameter("woT", [D, D], F8, isOutput=False)
    gwT = nc.declare_dram_parameter("gwT", [D, 2 * MLP], F8, isOutput=False)
    dwT = nc.declare_dram_parameter("dwT", [MLP, D], F8, isOutput=False)
    out_sl = nc.declare_dram_parameter("out_sl", [TOK, D], F32, isOutput=True)

    cc1_in = nc.dram_tensor("cc1_in", [B, 768], F32)
    cc1_out = nc.dram_tensor("cc1_out", [N_CORES * B, 768], F32, addr_space="Shared")
    cc2_in = nc.dram_tensor("cc2_in", [D], F32)
    cc2_out = nc.dram_tensor("cc2_out", [N_CORES, D], F32, addr_space="Shared")

    RG = [list(range(N_CORES))]

    with tile.TileContext(nc) as tc:
        import contextlib
        es = contextlib.ExitStack()
        with es:
            cst = es.enter_context(tc.tile_pool(name="cst", bufs=1))
            ps = es.enter_context(tc.tile_pool(name="ps", bufs=1, space="PSUM"))
            dr = es.enter_context(tc.tile_pool(name="dr", bufs=1, space="DRAM"))

            def pmm(tag="mm", bufs=2):
                return ps.tile([128, 512], F32, tag=tag, name=tag, bufs=bufs)

            # ---------------- consts ----------------
            identb = cst.tile([128, 128], BF)
            make_identity(nc, identb)
            identf = cst.tile([128, 128], F32)
            make_identity(nc, identf)
            ident8 = cst.tile([128, 128], F8)
            make_identity(nc, ident8)
            ones_row = cst.tile([1, 128], F32)
            nc.vector.memset(ones_row, 1.0)
            b15 = cst.tile([128, 1], F32)
            nc.vector.memset(b15, 15.0 * C_MAGIC / 16.0)
            mask_sb = cst.tile([N_CORES, 1], F32)
            nc.sync.dma_start(out=mask_sb, in_=mask8[:, :])
            bmask_sb = cst.tile([B, 1], F32)
            nc.sync.dma_start(out=bmask_sb, in_=bmask[:, :])
            gnr_sb = cst.tile([1, D], F32)
            nc.sync.dma_start(out=gnr_sb, in_=gnr[:, :])

            # long-lived small stat tiles
            q127A = cst.tile([128, 8], F32); dqA = cst.tile([128, 8], F32)
            dqAg = cst.tile([128, 8], F32)
            q127O = cst.tile([128, 8], F32); dqOo = cst.tile([128, 8], F32)
            q127C = cst.tile([128, 8], F32); dqCg = cst.tile([128, 8], F32)

            dqrow_d = dr.tile([D], F32, tag="dqrow")
            xnew_d = dr.tile([TOK, D], F32, tag="xnew")
            ca_d = dr.tile([TOK, TOK], F32, tag="cad")

            # pools with managed lifetimes
            pW1c = tc.tile_pool(name="pW1", bufs=1, side="right")   # wi/wf/wg/Sb  [P0..g-end]
            pW1 = pW1c.__enter__()
            pB1c = tc.tile_pool(name="pB1", bufs=1)   # B_* rows     [P0..C-end]
            pb1 = pB1c.__enter__()
            pXAc = tc.tile_pool(name="pXA", bufs=2)   # xa + LN1 [..A-end]
            pXA = pXAc.__enter__()
            pLNc = tc.tile_pool(name="pLN", bufs=2)   # adaln scratch [..bcast]
            pLN = pLNc.__enter__()

            # ---------------- adaln (sharded) + AllGather ----------------
            adb_sb = pLN.tile([1, 6 * D], F32, tag="adb", bufs=1)
            nc.sync.dma_start(out=adb_sb, in_=adb_row[:, :])
            c_sb = pLN.tile([128, 8, B], F32, tag="csb")
            nc.sync.dma_start(out=c_sb, in_=c_cols[:, :, :])
            cs_sb = pLN.tile([128, 8, B], F32, tag="cssb")
            nc.scalar.activation(out=cs_sb, in_=c_sb, func=AF.Silu)

            psA = pmm("mmf")[:B, :]
            psB = pmm("mmi")[:B, 0:256]
            for j in range(8):
                adw_j = pLN.tile([128, 768], F32, tag="adw")
                nc.sync.dma_start(out=adw_j, in_=adw_sl[128 * j:128 * (j + 1), :])
                nc.tensor.matmul(psA, cs_sb[:, j, :], adw_j[:, 0:512],
                                 start=(j == 0), stop=(j == 7))
                nc.tensor.matmul(psB, cs_sb[:, j, :], adw_j[:, 512:768],
                                 start=(j == 0), stop=(j == 7))
            ad_sb = pLN.tile([B, 768], F32, tag="adsb")
            nc.scalar.copy(out=ad_sb[:, 0:512], in_=psA)
            nc.scalar.copy(out=ad_sb[:, 512:768], in_=psB)
            nc.sync.dma_start(out=cc1_in[:, :], in_=ad_sb)
            nc.gpsimd.collective_compute(
                "AllGather", AL.bypass, ins=[cc1_in[:]], outs=[cc1_out[:]],
                replica_groups=RG)

            # ------- overlap collective: x load + weights + LN1 stats ------
            xa = pXA.tile([128, 8, D], F32, tag="xa", bufs=1)
            nc.sync.dma_start(out=xa,
                              in_=x_sl[:, :].rearrange("(i p) d -> p i d", p=128))
            wg_sb = pW1.tile([128, 8, D], F8, tag="wg")
            nc.sync.dma_start(out=wg_sb,
                              in_=wgT[:, :].rearrange("(a p) q -> p a q", p=128))
            wi_all = pW1.tile([128, 8, 8, 128], F8, tag="wi")
            nc.sync.dma_start(
                out=wi_all,
                in_=wiT[:, :].rearrange("(a p) (b q) -> p a b q", p=128, q=128))
            wf_all = pW1.tile([128, 8, 8, 128], F8, tag="wf")
            nc.sync.dma_start(
                out=wf_all,
                in_=wfT[:, :].rearrange("(a p) (b q) -> p a b q", p=128, q=128))
            Sb_i = pW1.tile([128, D], F32, tag="sbi")
            Sb_f = pW1.tile([128, D], F32, tag="sbf")

            muA = pXA.tile([128, 8], F32, tag="muA", bufs=1)
            varA = pXA.tile([128, 8], F32, tag="varA", bufs=1)
            for i in range(8):
                st = pXA.tile([128, 2, 6], F32, tag="bst")
                xr = xa[:, i, :].rearrange("p (s d) -> p s d", s=2)
                for s2 in range(2):
                    nc.vector.bn_stats(out=st[:, s2, :], in_=xr[:, s2, :])
                mv = pXA.tile([128, 2], F32, tag="bmv")
                nc.vector.bn_aggr(out=mv, in_=st)
                nc.vector.tensor_copy(out=muA[:, i:i + 1], in_=mv[:, 0:1])
                nc.vector.tensor_copy(out=varA[:, i:i + 1], in_=mv[:, 1:2])
            rstdLN = _rsqrt(nc, pXA, varA, 1.0, 1e-6, [128, 8], "rLN")
            nmr = pXA.tile([128, 8], F32, tag="nmr", bufs=1)
            nc.vector.tensor_tensor(out=nmr, in0=muA, in1=rstdLN, op=AL.mult)
            nc.vector.tensor_scalar(out=nmr, in0=nmr, scalar1=-1.0,
                                    scalar2=None, op0=AL.mult)

            # ------- collect adaln params + broadcast rows ----------
            params_sb = pLN.tile([1, 6 * D], F32, tag="params", bufs=1)
            for r in range(8):
                ag_r = pLN.tile([B, 768], F32, tag="ag1")
                nc.sync.dma_start(out=ag_r, in_=cc1_out[4 * r:4 * (r + 1), :])
                pp1 = pmm("mmf")[:1, :]
                pp2 = pmm("mmi")[:1, 0:256]
                nc.tensor.matmul(pp1, bmask_sb, ag_r[:, 0:512], start=True, stop=True)
                nc.tensor.matmul(pp2, bmask_sb, ag_r[:, 512:768], start=True, stop=True)
                nc.scalar.copy(out=params_sb[:, 768 * r:768 * r + 512], in_=pp1)
                nc.scalar.copy(out=params_sb[:, 768 * r + 512:768 * (r + 1)], in_=pp2)
            nc.vector.tensor_tensor(out=params_sb, in0=params_sb, in1=adb_sb,
                                    op=AL.add)

            def bcast_row(pool, row_ap, bname, plus1=False):
                t = pool.tile([128, D], F32, tag=bname, name=bname)
                for ch in range(0, D, 512):
                    pb = pmm("mm")
                    nc.tensor.matmul(pb, ones_row, row_ap[:, ch:ch + 512],
                                     start=True, stop=True)
                    if plus1:
                        nc.scalar.activation(out=t[:, ch:ch + 512], in_=pb,
                                             func=AF.Identity, bias=1.0)
                    else:
                        nc.scalar.copy(out=t[:, ch:ch + 512], in_=pb)
                return t

            pr = params_sb.rearrange("one (six d) -> one six d", six=6)
            B_sh1 = bcast_row(pb1, pr[:, 0, :], "Bsh1")
            B_sc1 = bcast_row(pb1, pr[:, 1, :], "Bsc1", plus1=True)
            B_g1 = bcast_row(pb1, pr[:, 2, :], "Bg1")
            B_sh2 = bcast_row(pb1, pr[:, 3, :], "Bsh2")
            B_sc2 = bcast_row(pb1, pr[:, 4, :], "Bsc2", plus1=True)
            B_g2 = bcast_row(cst, pr[:, 5, :], "Bg2")
            B_gn = bcast_row(pb1, gnr_sb, "Bgn")
            pLNc.__exit__(None, None, None)

            def quant_stats_sweep(src_get, n, amx, ssx, sb_pool, tagp):
                for i in range(n):
                    s = src_get(i)
                    nc.vector.tensor_reduce(out=amx[:, i:i + 1], in_=s, axis=AX.X,
                                            op=AL.max, apply_absolute_value=True)
                    scr = sb_pool.tile([128, s.free_size()], F32, bufs=1,
                                       tag=tagp + "sq", name=tagp + "sq")
                    nc.scalar.activation(out=scr, in_=s, func=AF.Square,
                                         accum_out=ssx[:, i:i + 1])

            def quant_batch(amx, ssx, dk, q127, dqt, dq_scaled, iws_scaled,
                            sb_pool, tagp):
                amc = sb_pool.tile([128, 8], F32, tag=tagp + "amc", name=tagp + "amc")
                nc.vector.tensor_scalar(out=amc, in0=amx, scalar1=1e-5,
                                        scalar2=None, op0=AL.max)
                rec = sb_pool.tile([128, 8], F32, tag=tagp + "rec", name=tagp + "rec")
                nc.vector.reciprocal(out=rec, in_=amc)
                nc.vector.tensor_scalar(out=q127, in0=rec, scalar1=127.0,
                                        scalar2=None, op0=AL.mult)
                rs = _rsqrt(nc, sb_pool, ssx, 1.0 / dk, 1e-8, [128, 8], tagp + "rs")
                nc.vector.tensor_tensor(out=dqt, in0=amc, in1=rs, op=AL.mult)
                nc.vector.tensor_scalar(out=dqt, in0=dqt, scalar1=1.0 / 127.0,
                                        scalar2=None, op0=AL.mult)
                if dq_scaled is not None:
                    nc.vector.tensor_scalar(out=dq_scaled, in0=dqt,
                                            scalar1=float(iws_scaled),
                                            scalar2=None, op0=AL.mult)

            cp_state = [0]

            def psum_copy(dst_ap, src_ap):
                k = cp_state[0] % 2
                cp_state[0] += 1
                if k == 0:
                    nc.scalar.copy(out=dst_ap, in_=src_ap)
                else:
                    nc.vector.tensor_copy(out=dst_ap, in_=src_ap)

            def quant_hilo(src, q_col, dst, i, sb_pool, tagp):
                """src [128, D] f32 (tokens on partitions), q_col [128,1];
                write fp8 (hi,lo) pairs transposed to dst[:, 2j/2j+1, 128i:]."""
                t2 = sb_pool.tile([128, D], F32, bufs=2, tag=tagp + "t2",
                                  name=tagp + "t2")
                nc.vector.tensor_scalar(out=t2, in0=src, scalar1=q_col,
                                        scalar2=C_MAGIC, op0=AL.mult, op1=AL.add)
                t3 = sb_pool.tile([128, D], F32, bufs=2, tag=tagp + "t3",
                                  name=tagp + "t3")
                nc.scalar.activation(out=t3, in_=t2, func=AF.Identity,
                                     scale=1.0 / 16.0, bias=b15[:, 0:1])
                hi8 = sb_pool.tile([128, D], BF, bufs=2, tag=tagp + "hi",
                                   name=tagp + "hi")
                nc.gpsimd.tensor_scalar(out=hi8, in0=t3, scalar1=16.0,
                                        scalar2=C16, op0=AL.mult, op1=AL.subtract)
                xq = sb_pool.tile([128, D], BF, bufs=2, tag=tagp + "xq",
                                  name=tagp + "xq")
                nc.vector.tensor_scalar(out=xq, in0=t2, scalar1=C_MAGIC,
                                        scalar2=None, op0=AL.subtract)
                lo8 = sb_pool.tile([128, D], BF, bufs=2, tag=tagp + "lo",
                                   name=tagp + "lo")
                nc.vector.scalar_tensor_tensor(out=lo8, in0=hi8, scalar=-1.0,
                                               in1=xq, op0=AL.mult, op1=AL.add)
                for g4 in range(0, 8, 4):
                    tph = ps.tile([128, 512], BF, tag="tpx", name="tpx", bufs=2)
                    for jj in range(4):
                        jb = g4 + jj
                        nc.tensor.transpose(tph[:, 128 * jj:128 * (jj + 1)],
                                            hi8[:, 128 * jb:128 * (jb + 1)], identb)
                    psum_copy(dst[:, 2 * g4:2 * g4 + 8:2, 128 * i:128 * (i + 1)],
                              tph[:].rearrange("p (a q) -> p a q", a=4))
                    tpl = ps.tile([128, 512], BF, tag="tpx", name="tpx", bufs=2)
                    for jj in range(4):
                        jb = g4 + jj
                        nc.tensor.transpose(tpl[:, 128 * jj:128 * (jj + 1)],
                                            lo8[:, 128 * jb:128 * (jb + 1)], identb)
                    psum_copy(dst[:, 2 * g4 + 1:2 * g4 + 8:2,
                                  128 * i:128 * (i + 1)],
                              tpl[:].rearrange("p (a q) -> p a q", a=4))

            # ---------------- phase A: LN + modulate + quant ----------------
            pXQc = tc.tile_pool(name="pXQ", bufs=1, side="right")
            pXQ = pXQc.__enter__()
            xqT = pXQ.tile([128, 16, D], F8)
            with tc.tile_pool(name="pa", bufs=2) as pa:
                amA = pa.tile([128, 8], F32, tag="amA")
                ssA = pa.tile([128, 8], F32, tag="ssA")
                moda = pa.tile([128, 8, D], F32, tag="moda", bufs=1)
                for i in range(8):
                    u = pa.tile([128, D], F32, tag="u", bufs=2)
                    nc.scalar.activation(out=u, in_=xa[:, i, :], func=AF.Identity,
                                         scale=rstdLN[:, i:i + 1],
                                         bias=nmr[:, i:i + 1])
                    tt = pa.tile([128, D], F32, tag="tt", bufs=2)
                    nc.vector.tensor_tensor(out=tt, in0=u, in1=B_sc1, op=AL.mult)
                    nc.gpsimd.tensor_tensor(out=moda[:, i, :], in0=tt, in1=B_sh1,
                                            op=AL.add)
                quant_stats_sweep(lambda i: moda[:, i, :], 8, amA, ssA, pa, "qa")
                quant_batch(amA, ssA, D, q127A, dqA, dqAg, iw["g"], pa, "qa")
                for i in range(8):
                    nc.sync.dma_start(
                        out=dqrow_d[128 * i:128 * (i + 1)].rearrange(
                            "(p one) -> p one", one=1),
                        in_=dqA[:, i:i + 1])
                dqrow_sb = pa.tile([1, D], F32, tag="dqrow")
                nc.sync.dma_start(out=dqrow_sb,
                                  in_=dqrow_d[:].rearrange("(one d) -> one d", one=1))
                oi = pa.tile([1, 128], F32, tag="oi")
                nc.vector.memset(oi, float(iw["i"]))
                of = pa.tile([1, 128], F32, tag="of")
                nc.vector.memset(of, float(iw["f"]))
                for ch in range(0, D, 512):
                    pb = pmm("mm")
                    nc.tensor.matmul(pb, oi, dqrow_sb[:, ch:ch + 512],
                                     start=True, stop=True)
                    nc.scalar.copy(out=Sb_i[:, ch:ch + 512], in_=pb)
                    pb2 = pmm("mm")
                    nc.tensor.matmul(pb2, of, dqrow_sb[:, ch:ch + 512],
                                     start=True, stop=True)
                    nc.vector.tensor_copy(out=Sb_f[:, ch:ch + 512], in_=pb2)
                for i in range(8):
                    quant_hilo(moda[:, i, :], q127A[:, i:i + 1], xqT, i, pa, "ra")
            pXAc.__exit__(None, None, None)

            # ---------------- phase B: i/f matmuls + scan ----------------
            pSGc = tc.tile_pool(name="pSG", bufs=1)   # gs + hT [B..o-end]
            pSG = pSGc.__enter__()
            gs = pSG.tile([128, 8, D], F32, tag="gs")
            hT = pSG.tile([128, 8, D], F32, tag="hT")
            pHAc = tc.tile_pool(name="pHA", bufs=1)   # ha [scan..fixup]
            pHA = pHAc.__enter__()
            ha = pHA.tile([128, 8, TOK], F32)
            pbsc = tc.tile_pool(name="pbs", bufs=1)   # scan scratch
            pb = pbsc.__enter__()
            for m in range(8):
                ft = pb.tile([128, TOK], F32, tag="ftm")
                it = pb.tile([128, TOK], F32, tag="itm")
                for cki, ck in enumerate(range(0, TOK, 512)):
                    pf = pmm("mmf")
                    pi = pmm("mmi")
                    for j in range(8):
                        nc.tensor.matmul(pf, _wpair(wf_all[:, j, m, :]),
                                         xqT[:, 2 * j:2 * j + 2, ck:ck + 512],
                                         start=(j == 0), stop=(j == 7),
                                         perf_mode=PM.DoubleRow)
                    for j in range(8):
                        nc.tensor.matmul(pi, _wpair(wi_all[:, j, m, :]),
                                         xqT[:, 2 * j:2 * j + 2, ck:ck + 512],
                                         start=(j == 0), stop=(j == 7),
                                         perf_mode=PM.DoubleRow)
                    nc.vector.tensor_tensor(out=ft[:, ck:ck + 512], in0=pf,
                                            in1=Sb_f[:, ck:ck + 512], op=AL.mult)
                    nc.vector.tensor_tensor(out=it[:, ck:ck + 512], in0=pi,
                                            in1=Sb_i[:, ck:ck + 512], op=AL.mult)
                sigf = pb.tile([128, TOK], F32, tag="sigf")
                nc.scalar.activation(out=sigf, in_=ft, func=AF.Sigmoid)
                sili = pb.tile([128, TOK], F32, tag="sili")
                nc.scalar.activation(out=sili, in_=it, func=AF.Silu)
                omf = pb.tile([128, TOK], F32, tag="omf")
                nc.gpsimd.tensor_scalar(out=omf, in0=sigf, scalar1=-1.0,
                                        scalar2=1.0, op0=AL.mult, op1=AL.add)
                ifin = pb.tile([128, TOK], F32, tag="ifin")
                nc.vector.tensor_tensor(out=ifin, in0=sili, in1=omf, op=AL.mult)
                nc.vector.tensor_tensor_scan(ha[:, m, :], sigf, ifin, 0.0,
                                             op0=AL.mult, op1=AL.add)
                cam = pb.tile([128, TOK], F32, tag="cam", bufs=1)
                nc.vector.tensor_tensor_scan(cam, sigf, sigf, 1.0,
                                             op0=AL.mult, op1=AL.bypass)
                nc.sync.dma_start(out=ca_d[128 * m:128 * (m + 1), :], in_=cam)
            nc.sync.dma_start(
                out=cc2_in[:].rearrange("(a p) -> p a", p=128),
                in_=ha[:, :, TOK - 1:TOK].rearrange("p a one -> p (a one)"))
            nc.gpsimd.collective_compute(
                "AllGather", AL.bypass, ins=[cc2_in[:]], outs=[cc2_out[:]],
                replica_groups=RG)
            pbsc.__exit__(None, None, None)

            # ---- overlap the collective: g = silu(xq @ wg) * gnorm ----
            pb2c = tc.tile_pool(name="pb2", bufs=1)   # g + fixup scratch
            pb2 = pb2c.__enter__()
            for t in range(8):
                for ck in range(0, D, 512):
                    pg = pmm("mm")
                    for j in range(8):
                        nc.tensor.matmul(pg, xqT[:, 2 * j:2 * j + 2,
                                                 128 * t:128 * (t + 1)],
                                         _wpair(wg_sb[:, j, ck:ck + 512]),
                                         start=(j == 0), stop=(j == 7),
                                         perf_mode=PM.DoubleRow)
                    scr = pb2.tile([128, 512], F32, tag="gscr", bufs=2)
                    nc.scalar.activation(out=scr, in_=pg, func=AF.Silu,
                                         scale=dqAg[:, t:t + 1])
                    nc.gpsimd.tensor_tensor(out=gs[:, t, ck:ck + 512], in0=scr,
                                            in1=B_gn[:, ck:ck + 512], op=AL.mult)
            pXQc.__exit__(None, None, None)
            pW1c.__exit__(None, None, None)

            # ---- carry fixup + transpose h -> hT ----
            ag2 = pb2.tile([N_CORES, D], F32, tag="ag2")
            nc.sync.dma_start(out=ag2, in_=cc2_out[:, :])
            for m in range(8):
                pc0 = pmm("mm")[:, 0:1]
                nc.tensor.matmul(pc0, ag2[:, 128 * m:128 * (m + 1)], mask_sb,
                                 start=True, stop=True)
                carry = pb2.tile([128, 1], F32, tag="carry", bufs=2)
                nc.scalar.copy(out=carry, in_=pc0)
                cam2 = pb2.tile([128, TOK], F32, tag="cam2", bufs=2)
                nc.sync.dma_start(out=cam2, in_=ca_d[128 * m:128 * (m + 1), :])
                hfix = pb2.tile([128, TOK], F32, tag="hfix", bufs=2)
                nc.vector.scalar_tensor_tensor(out=hfix, in0=cam2,
                                               scalar=carry, in1=ha[:, m, :],
                                               op0=AL.mult, op1=AL.add)
                for g4 in range(0, 8, 4):
                    tp = pmm("mm")
                    for jj in range(4):
                        t_i = g4 + jj
                        nc.tensor.transpose(tp[:, 128 * jj:128 * (jj + 1)],
                                            hfix[:, 128 * t_i:128 * (t_i + 1)],
                                            identf)
                    psum_copy(hT[:, g4:g4 + 4, 128 * m:128 * (m + 1)],
                              tp[:].rearrange("p (a q) -> p a q", a=4))
            pb2c.__exit__(None, None, None)
            pHAc.__exit__(None, None, None)

            # ---------------- o-stage ----------------
            pX2c = tc.tile_pool(name="pX2", bufs=1, side="right")
            x2qT = pX2c.__enter__().tile([128, 16, D], F8)
            pOQc = tc.tile_pool(name="pOQ", bufs=1, side="right")
            pOQ = pOQc.__enter__()
            oqT = pOQ.tile([128, 16, D], F8)
            with tc.tile_pool(name="po", bufs=2) as po:
                mshA = po.tile([128, 8, 16], F32, tag="msh", bufs=1)
                for t in range(8):
                    sq = po.tile([128, D], F32, tag="sq", bufs=2)
                    nc.gpsimd.tensor_tensor(out=sq, in0=hT[:, t, :],
                                            in1=hT[:, t, :], op=AL.mult)
                    nc.vector.tensor_reduce(
                        out=mshA[:, t, :],
                        in_=sq.rearrange("p (h d) -> p h d", h=NH),
                        axis=AX.X, op=AL.add)
                rstdH = _rsqrt(nc, po,
                               mshA[:, :, :].rearrange("p a b -> p (a b)"),
                               1.0 / HD, 1e-5, [128, 128], "rH")
                rH = rstdH.rearrange("p (a b) -> p a b", a=8)
                amO = po.tile([128, 8], F32, tag="amO")
                ssO = po.tile([128, 8], F32, tag="ssO")
                for t in range(8):
                    hn = po.tile([128, D], F32, tag="hn", bufs=2)
                    rb = bass.AP(tensor=rH.tensor, offset=rH[:, t, :].offset,
                                 ap=[rH.ap[0], [1, NH], [0, HD]])
                    nc.vector.tensor_tensor(
                        out=hn.rearrange("p (h d) -> p h d", h=NH),
                        in0=hT[:, t, :].rearrange("p (h d) -> p h d", h=NH),
                        in1=rb, op=AL.mult)
                    # oa overwrites gs in place
                    nc.gpsimd.tensor_tensor(out=gs[:, t, :], in0=hn,
                                            in1=gs[:, t, :], op=AL.mult)
                quant_stats_sweep(lambda t: gs[:, t, :], 8, amO, ssO, po, "qo")
                quant_batch(amO, ssO, D, q127O, dqOo, dqOo, 1.0, po, "qo")
                nc.vector.tensor_scalar(out=dqOo, in0=dqOo, scalar1=float(iw["o"]),
                                        scalar2=None, op0=AL.mult)
                for t in range(8):
                    quant_hilo(gs[:, t, :], q127O[:, t:t + 1], oqT, t, po, "ro")
            pSGc.__exit__(None, None, None)

            # ---------------- phase C: wo matmul + residual + LN2 ----------
            pGWc = tc.tile_pool(name="pGW", bufs=1, side="right")
            pDWc = tc.tile_pool(name="pDW", bufs=1, side="right")
            pM2c = tc.tile_pool(name="pM2", bufs=1)
            mod2 = pM2c.__enter__().tile([128, 8, D], F32)
            with tc.tile_pool(name="pc", bufs=2) as pc:
                wo_sb = pc.tile([128, 8, D], F8, tag="wosb", bufs=1)
                nc.sync.dma_start(out=wo_sb,
                                  in_=woT[:, :].rearrange("(a p) q -> p a q", p=128))
                muC = pc.tile([128, 8], F32, tag="muC")
                varC = pc.tile([128, 8], F32, tag="varC")
                for t in range(8):
                    xa2 = pc.tile([128, D], F32, tag="xa2", bufs=2)
                    nc.sync.dma_start(out=xa2, in_=x_sl[128 * t:128 * (t + 1), :])
                    xn = pc.tile([128, D], F32, tag="xn", bufs=2)
                    for ck in range(0, D, 512):
                        pw = pmm("mmf")
                        for j in range(8):
                            nc.tensor.matmul(pw, oqT[:, 2 * j:2 * j + 2,
                                                     128 * t:128 * (t + 1)],
                                             _wpair(wo_sb[:, j, ck:ck + 512]),
                                             start=(j == 0), stop=(j == 7),
                                             perf_mode=PM.DoubleRow)
                        at = pc.tile([128, 512], F32, tag="at", bufs=2)
                        nc.vector.tensor_scalar(out=at, in0=pw,
                                                scalar1=dqOo[:, t:t + 1],
                                                scalar2=None, op0=AL.mult)
                        ug = pc.tile([128, 512], F32, tag="ug", bufs=2)
                        nc.gpsimd.tensor_tensor(out=ug, in0=at,
                                                in1=B_g1[:, ck:ck + 512], op=AL.mult)
                        nc.vector.tensor_tensor(out=xn[:, ck:ck + 512], in0=ug,
                                                in1=xa2[:, ck:ck + 512], op=AL.add)
                    nc.sync.dma_start(out=xnew_d[128 * t:128 * (t + 1), :], in_=xn)
                    st = pc.tile([128, 2, 6], F32, tag="bst2")
                    xrr = xn.rearrange("p (s d) -> p s d", s=2)
                    for s2 in range(2):
                        nc.vector.bn_stats(out=st[:, s2, :], in_=xrr[:, s2, :])
                    mv = pc.tile([128, 2], F32, tag="bmv2")
                    nc.vector.bn_aggr(out=mv, in_=st)
                    nc.vector.tensor_copy(out=muC[:, t:t + 1], in_=mv[:, 0:1])
                    nc.vector.tensor_copy(out=varC[:, t:t + 1], in_=mv[:, 1:2])
                # wo matmuls done -> free oqT, start gw load
                pOQc.__exit__(None, None, None)
                gw_sb = pGWc.__enter__().tile([128, 8, 2 * MLP], F8)
                nc.sync.dma_start(
                    out=gw_sb,
                    in_=gwT[:, :].rearrange("(a p) q -> p a q", p=128))
                rstdC = _rsqrt(nc, pc, varC, 1.0, 1e-6, [128, 8], "rC")
                nmrC = pc.tile([128, 8], F32, tag="nmrC")
                nc.vector.tensor_tensor(out=nmrC, in0=muC, in1=rstdC, op=AL.mult)
                nc.vector.tensor_scalar(out=nmrC, in0=nmrC, scalar1=-1.0,
                                        scalar2=None, op0=AL.mult)
                for t in range(8):
                    xn2 = pc.tile([128, D], F32, tag="xn2", bufs=2)
                    nc.sync.dma_start(out=xn2, in_=xnew_d[128 * t:128 * (t + 1), :])
                    u2 = pc.tile([128, D], F32, tag="u2", bufs=1)
                    nc.scalar.activation(out=u2, in_=xn2, func=AF.Identity,
                                         scale=rstdC[:, t:t + 1],
                                         bias=nmrC[:, t:t + 1])
                    tt2 = pc.tile([128, D], F32, tag="tt2", bufs=1)
                    nc.vector.tensor_tensor(out=tt2, in0=u2, in1=B_sc2, op=AL.mult)
                    nc.gpsimd.tensor_tensor(out=mod2[:, t, :], in0=tt2, in1=B_sh2,
                                            op=AL.add)
            with tc.tile_pool(name="pcq", bufs=2) as pcq:
                amC = pcq.tile([128, 8], F32, tag="amC")
                ssC = pcq.tile([128, 8], F32, tag="ssC")
                quant_stats_sweep(lambda t: mod2[:, t, :], 8, amC, ssC, pcq, "qc")
                quant_batch(amC, ssC, D, q127C, dqCg, dqCg, iw["gate"], pcq, "qc")
                for t in range(8):
                    quant_hilo(mod2[:, t, :], q127C[:, t:t + 1], x2qT, t, pcq, "rc")
            pM2c.__exit__(None, None, None)
            pB1c.__exit__(None, None, None)

            # ---------------- phase D: MLP (SBUF-resident, pipelined) ------
            dw_sb = pDWc.__enter__().tile([128, 32, D], F8)
            nc.sync.dma_start(out=dw_sb,
                              in_=dwT[:, :].rearrange("(a p) q -> p a q", p=128))
            with tc.tile_pool(name="pd", bufs=2) as pd:
                def emit_gate_half(t, h2_h, amD_t, ssD_t, half):
                    for ck in range(4):
                        c0 = 2048 * half + 512 * ck
                        pg = pmm("mmf")
                        py = pmm("mmi")
                        for j in range(8):
                            nc.tensor.matmul(pg, x2qT[:, 2 * j:2 * j + 2,
                                                      128 * t:128 * (t + 1)],
                                             _wpair(gw_sb[:, j, c0:c0 + 512]),
                                             start=(j == 0), stop=(j == 7),
                                             perf_mode=PM.DoubleRow)
                        for j in range(8):
                            nc.tensor.matmul(py, x2qT[:, 2 * j:2 * j + 2,
                                                      128 * t:128 * (t + 1)],
                                             _wpair(gw_sb[:, j,
                                                          MLP + c0:MLP + c0 + 512]),
                                             start=(j == 0), stop=(j == 7),
                                             perf_mode=PM.DoubleRow)
                        sil = pd.tile([128, 512], F32, tag="sil", bufs=2)
                        nc.scalar.activation(out=sil, in_=pg, func=AF.Silu,
                                             scale=dqCg[:, t:t + 1])
                        hc = 512 * ck
                        nc.vector.scalar_tensor_tensor(
                            out=h2_h[:, hc:hc + 512], in0=py,
                            scalar=dqCg[:, t:t + 1], in1=sil,
                            op0=AL.mult, op1=AL.mult)
                        cki = 4 * half + ck
                        nc.vector.tensor_reduce(out=amD_t[:, cki:cki + 1],
                                                in_=h2_h[:, hc:hc + 512],
                                                axis=AX.X, op=AL.max,
                                                apply_absolute_value=True)
                        scr = pd.tile([128, 512], F32, tag="sqd", bufs=2)
                        nc.scalar.activation(out=scr, in_=h2_h[:, hc:hc + 512],
                                             func=AF.Square,
                                             accum_out=ssD_t[:, cki:cki + 1])

                def emit_quant_h2(t, h2_hs, amD_t, ssD_t, h2q_t):
                    amD = pd.tile([128, 1], F32, tag="amD", bufs=2)
                    nc.vector.tensor_reduce(out=amD, in_=amD_t, axis=AX.X,
                                            op=AL.max)
                    ssD = pd.tile([128, 1], F32, tag="ssD", bufs=2)
                    nc.vector.tensor_reduce(out=ssD, in_=ssD_t, axis=AX.X,
                                            op=AL.add)
                    amc = pd.tile([128, 1], F32, tag="qdamc", bufs=2)
                    nc.vector.tensor_scalar(out=amc, in0=amD, scalar1=1e-5,
                                            scalar2=None, op0=AL.max)
                    rec = pd.tile([128, 1], F32, tag="qdrec", bufs=2)
                    nc.vector.reciprocal(out=rec, in_=amc)
                    q127 = pd.tile([128, 1], F32, tag="qdq", bufs=2)
                    nc.vector.tensor_scalar(out=q127, in0=rec, scalar1=127.0,
                                            scalar2=None, op0=AL.mult)
                    rs = _rsqrt(nc, pd, ssD, 1.0 / MLP, 1e-8, [128, 1], "rD")
                    dq = pd.tile([128, 1], F32, tag="qddq", bufs=2)
                    nc.vector.tensor_tensor(out=dq, in0=amc, in1=rs, op=AL.mult)
                    nc.vector.tensor_scalar(out=dq, in0=dq,
                                            scalar1=float(iw["down"]) / 127.0,
                                            scalar2=None, op0=AL.mult)
                    for half in range(2):
                        h2_h = h2_hs[half]
                        nc.vector.tensor_scalar(out=h2_h, in0=h2_h, scalar1=q127,
                                                scalar2=C_MAGIC, op0=AL.mult,
                                                op1=AL.add)
                        kq = pd.tile([128, 2048], BF, tag="kq", bufs=2)
                        nc.gpsimd.tensor_scalar(out=kq, in0=h2_h, scalar1=C_MAGIC,
                                                scalar2=None, op0=AL.subtract)
                        for g8 in range(0, 16, 4):
                            tp = ps.tile([128, 512], BF, tag="tpx", name="tpx",
                                         bufs=2)
                            for jj in range(4):
                                j2 = g8 + jj
                                nc.tensor.transpose(
                                    tp[:, 128 * jj:128 * (jj + 1)],
                                    kq[:, 128 * j2:128 * (j2 + 1)], identb)
                            psum_copy(h2q_t[:, 16 * half + g8:16 * half + g8 + 4, :],
                                      tp[:].rearrange("p (a q) -> p a q", a=4))
                    return dq

                def emit_down(t, h2q_t, dq):
                    xn3 = pd.tile([128, D], F32, tag="xn3", bufs=1)
                    nc.sync.dma_start(out=xn3,
                                      in_=xnew_d[128 * t:128 * (t + 1), :])
                    outt = pd.tile([128, D], F32, tag="outt", bufs=2)
                    for ck in range(0, D, 512):
                        pdn = pmm("mm")
                        for j2 in range(32):
                            nc.tensor.matmul(pdn, h2q_t[:, j2, :],
                                             dw_sb[:, j2, ck:ck + 512],
                                             start=(j2 == 0), stop=(j2 == 31))
                        u2 = pd.tile([128, 512], F32, tag="u2d", bufs=2)
                        nc.vector.tensor_scalar(out=u2, in0=pdn, scalar1=dq,
                                                scalar2=None, op0=AL.mult)
                        v2 = pd.tile([128, 512], F32, tag="v2d", bufs=2)
                        nc.gpsimd.tensor_tensor(out=v2, in0=u2,
                                                in1=B_g2[:, ck:ck + 512], op=AL.mult)
                        nc.vector.tensor_tensor(out=outt[:, ck:ck + 512], in0=v2,
                                                in1=xn3[:, ck:ck + 512], op=AL.add)
                    nc.sync.dma_start(out=out_sl[128 * t:128 * (t + 1), :], in_=outt)

                prev = None
                for t in range(8):
                    h2_a = pd.tile([128, 2048], F32, tag="h2h", bufs=4)
                    h2_b = pd.tile([128, 2048], F32, tag="h2h", bufs=4)
                    amD_t = pd.tile([128, 8], F32, tag="amDt", bufs=2)
                    ssD_t = pd.tile([128, 8], F32, tag="ssDt", bufs=2)
                    emit_gate_half(t, h2_a, amD_t, ssD_t, 0)
                    emit_gate_half(t, h2_b, amD_t, ssD_t, 1)
                    if prev is not None:
                        pt, ph2s, pam, pss = prev
                        h2q_t = pd.tile([128, 32, 128], BF, tag="h2qt", bufs=2)
                        dq = emit_quant_h2(pt, ph2s, pam, pss, h2q_t)
                        emit_down(pt, h2q_t, dq)
                    prev = (t, (h2_a, h2_b), amD_t, ssD_t)
                pt, ph2s, pam, pss = prev
                h2q_t = pd.tile([128, 32, 128], BF, tag="h2qt", bufs=2)
                dq = emit_quant_h2(pt, ph2s, pam, pss, h2q_t)
                emit_down(pt, h2q_t, dq)
            pDWc.__exit__(None, None, None)
            pGWc.__exit__(None, None, None)
            pX2c.__exit__(None, None, None)

    nc.finalize()
    return nc


@functools.lru_cache(maxsize=2)
def _build_cached(iw_items):
    return _build(dict(iw_items))


def kernel(x, c, adaln_w, adaln_b, wi, wf, wg, gnorm_w, wo, gate_w, down_w):
    x = np.ascontiguousarray(np.asarray(x, dtype=np.float32))
    c = np.ascontiguousarray(np.asarray(c, dtype=np.float32))
    adaln_w = np.asarray(adaln_w, dtype=np.float32)
    adaln_b = np.asarray(adaln_b, dtype=np.float32)
    gnorm_w = np.asarray(gnorm_w, dtype=np.float32)

    mi, iwi = _quant_w(np.asarray(wi, dtype=np.float32))
    mf, iwf = _quant_w(np.asarray(wf, dtype=np.float32))
    mg, iwg = _quant_w(np.asarray(wg, dtype=np.float32))
    mo, iwo = _quant_w(np.asarray(wo, dtype=np.float32))
    mgate, iwgate = _quant_w(np.asarray(gate_w, dtype=np.float32))
    mdown, iwdown = _quant_w(np.asarray(down_w, dtype=np.float32))

    iw = {"i": float(iwi), "f": float(iwf), "g": float(iwg), "o": float(iwo),
          "gate": float(iwgate), "down": float(iwdown)}
    nc = _build_cached(tuple(sorted(iw.items())))

    wiT_h = np.ascontiguousarray(mi.T)
    wfT_h = np.ascontiguousarray(mf.T)
    wgT_h = np.ascontiguousarray(mg.T)
    woT_h = np.ascontiguousarray(mo.T)
    gwT_h = np.ascontiguousarray(mgate.T)
    dwT_h = np.ascontiguousarray(mdown.T)
    adwT = np.ascontiguousarray(adaln_w.T)          # [D, 6D]
    adb_row_h = np.ascontiguousarray(adaln_b[None, :])
    gnr_h = np.ascontiguousarray(np.tile(gnorm_w, NH)[None, :])
    c_cols_h = np.ascontiguousarray(
        c.T.reshape(8, 128, B).transpose(1, 0, 2))   # [128, 8, B]

    in_maps = []
    for core in range(N_CORES):
        b, half = core // 2, core % 2
        mask = np.zeros((N_CORES, 1), np.float32)
        if half == 1:
            mask[core - 1, 0] = 1.0
        bm = np.zeros((B, 1), np.float32)
        bm[b, 0] = 1.0
        in_maps.append({
            "x_sl": np.ascontiguousarray(x[b, half * TOK:(half + 1) * TOK, :]),
            "c_cols": c_cols_h,
            "adw_sl": np.ascontiguousarray(adwT[:, 768 * core:768 * (core + 1)]),
            "adb_row": adb_row_h,
            "mask8": mask,
            "bmask": bm,
            "gnr": gnr_h,
            "wiT": wiT_h, "wfT": wfT_h, "wgT": wgT_h, "woT": woT_h,
            "gwT": gwT_h, "dwT": dwT_h,
        })

    res = run_bass_kernel_spmd(nc, in_maps, core_ids=list(range(N_CORES)))
    out = np.zeros((B, T, D), np.float32)
    for core in range(N_CORES):
        b, half = core // 2, core % 2
        out[b, half * TOK:(half + 1) * TOK, :] = res.results[core]["out_sl"]
    return out


# revision 21
# speedup vs baseline: 1.8330x; 1.8330x over previous
"""Trainium2 Bass kernel for nn_DiTBlock (HGRN-attention DiT block).

Sharding: 8 cores = 4 batches x 2 half-sequences (1024 tokens each).
All big matmuls run as EXACT integer arithmetic on the fp8 PE path:
activations are int8-grid quantized (matching the reference bitlinear),
then split x = hi + lo with hi = 16*round(x/16) (multiples of 16, fp8-
exact) and lo = x - hi (|lo| <= 8, fp8-exact). A DoubleRow fp8 matmul
contracts the (hi, lo) pair against a stride-0-replicated ternary fp8
weight pair in one instruction -> 2x the bf16 matmul rate with
bit-identical results. The down-projection uses bf16 activations x fp8
weights (plain matmul) to keep the 8M-element h2 quant cheap.
The time-recurrence h_t = f_t*h_{t-1} + i_t uses the DVE
tensor_tensor_scan; the half-sequence boundary carry crosses cores via
AllGather + one-hot mask matmul. adaln params are computed on-device,
sharded 8 ways over the 6144 outputs and AllGathered.
"""
import functools
import numpy as np
import ml_dtypes

import concourse.bass as bass
import concourse.bacc as bacc_mod
import concourse.mybir as mybir
import concourse.tile as tile
from concourse.masks import make_identity
from concourse.bass_utils import run_bass_kernel_spmd

E4M3 = ml_dtypes.float8_e4m3
F32 = mybir.dt.float32
BF = mybir.dt.bfloat16
F8 = mybir.dt.float8e4
U32 = mybir.dt.uint32
AL = mybir.AluOpType
AF = mybir.ActivationFunctionType
AX = mybir.AxisListType
PM = mybir.MatmulPerfMode

B, T, D = 4, 2048, 1024
TOK = 1024          # tokens per core
NH, HD = 16, 64
MLP = 4096
N_CORES = 8
C_MAGIC = float(1.5 * 2 ** 23)
C16 = 16.0 * C_MAGIC
MAGIC_U32 = 0x5F3759DF


def _quant_w(w):
    invws = float(np.clip(np.abs(w).mean(dtype=np.float64), 1e-5, None))
    m = np.clip(np.round(w.astype(np.float64) / invws), -1, 1).astype(np.float32)
    return np.ascontiguousarray(m.astype(E4M3)), np.float32(invws)


def _rsqrt(nc, sb, x_ap, scale, bias, shape, tag):
    """out = rsqrt(x*scale + bias), Newton on DVE. Returns a new tile."""
    t = sb.tile(shape, F32, tag=tag + "_t", name=tag + "_t")
    nc.vector.tensor_scalar(out=t, in0=x_ap, scalar1=float(scale),
                            scalar2=float(bias), op0=AL.mult, op1=AL.add)
    y = sb.tile(shape, F32, tag=tag + "_y", name=tag + "_y")
    sh = sb.tile(shape, F32, tag=tag + "_s", name=tag + "_s")
    nc.vector.tensor_scalar(out=sh[:].bitcast(U32), in0=t[:].bitcast(U32),
                            scalar1=1, scalar2=None, op0=AL.logical_shift_right)
    mg = sb.tile(shape, F32, tag=tag + "_m", name=tag + "_m")
    nc.vector.memset(mg[:].bitcast(U32), MAGIC_U32)
    nc.vector.tensor_tensor(out=y[:].bitcast(U32), in0=mg[:].bitcast(U32),
                            in1=sh[:].bitcast(U32), op=AL.subtract)
    e = sb.tile(shape, F32, tag=tag + "_e", name=tag + "_e")
    for _ in range(3):
        nc.vector.tensor_tensor(out=e, in0=y, in1=y, op=AL.mult)
        nc.vector.tensor_tensor(out=e, in0=e, in1=t, op=AL.mult)
        nc.vector.tensor_scalar(out=e, in0=e, scalar1=-0.5, scalar2=1.5,
                                op0=AL.mult, op1=AL.add)
        nc.vector.tensor_tensor(out=y, in0=y, in1=e, op=AL.mult)
    return y


def _wpair(ap2):
    """[128, X] weight AP -> [128, 2, X] with stride-0 pair dim."""
    return bass.AP(tensor=ap2.tensor, offset=ap2.offset,
                   ap=[ap2.ap[0], [0, 2], ap2.ap[-1]])


def _build(iw, gn_ones=True):
    GN_ONES = gn_ones
    nc = bacc_mod.Bacc("TRN2", target_bir_lowering=False)

    x_sl = nc.declare_dram_parameter("x_sl", [TOK, D], F32, isOutput=False)
    c_cols = nc.declare_dram_parameter("c_cols", [128, 8, B], F32, isOutput=False)
    adw_sl = nc.declare_dram_parameter("adw_sl", [D, 768], F32, isOutput=False)
    adb_row = nc.declare_dram_parameter("adb_row", [1, 6 * D], F32, isOutput=False)
    mask8 = nc.declare_dram_parameter("mask8", [N_CORES, 1], F32, isOutput=False)
    bmask = nc.declare_dram_parameter("bmask", [B, 1], F32, isOutput=False)
    gnr = nc.declare_dram_parameter("gnr", [1, D], F32, isOutput=False)
    wiT = nc.declare_dram_parameter("wiT", [D, D], F8, isOutput=False)
    wfT = nc.declare_dram_parameter("wfT", [D, D], F8, isOutput=False)
    wgT = nc.declare_dram_parameter("wgT", [D, D], F8, isOutput=False)
    woT = nc.declare_dram_parameter("woT", [D, D], F8, isOutput=False)
    gwT = nc.declare_dram_parameter("gwT", [D, 2 * MLP], F8, isOutput=False)
    dwT = nc.declare_dram_parameter("dwT", [MLP, D], F8, isOutput=False)
    out_sl = nc.declare_dram_parameter("out_sl", [TOK, D], F32, isOutput=True)

    cc1_in = nc.dram_tensor("cc1_in", [B, 768], F32)
    cc1_out = nc.dram_tensor("cc1_out", [N_CORES * B, 768], F32, addr_space="Shared")
    cc2_in = nc.dram_tensor("cc2_in", [D], F32)
    cc2_out = nc.dram_tensor("cc2_out", [N_CORES, D], F32, addr_space="Shared")

    RG = [list(range(N_CORES))]

    with tile.TileContext(nc) as tc:
        import contextlib
        es = contextlib.ExitStack()
        with es:
            cst = es.enter_context(tc.tile_pool(name="cst", bufs=1))
            ps = es.enter_context(tc.tile_pool(name="ps", bufs=1, space="PSUM"))
            dr = es.enter_context(tc.tile_pool(name="dr", bufs=1, space="DRAM"))

            def pmm(tag="mm", bufs=2):
                return ps.tile([128, 512], F32, tag=tag, name=tag, bufs=bufs)

            # ---------------- consts ----------------
            identb = cst.tile([128, 128], BF)
            make_identity(nc, identb)
            identf = cst.tile([128, 128], F32)
            make_identity(nc, identf)
            ident8 = cst.tile([128, 128], F8)
            make_identity(nc, ident8)
            ones_row = cst.tile([1, 128], F32)
            nc.vector.memset(ones_row, 1.0)
            cC = cst.tile([128, 1], F32)
            nc.vector.memset(cC, C_MAGIC)
            mask_sb = cst.tile([N_CORES, 1], F32)
            nc.sync.dma_start(out=mask_sb, in_=mask8[:, :])
            bmask_sb = cst.tile([B, 1], F32)
            nc.sync.dma_start(out=bmask_sb, in_=bmask[:, :])
            gnr_sb = cst.tile([1, D], F32)
            nc.sync.dma_start(out=gnr_sb, in_=gnr[:, :])

            # long-lived small stat tiles
            q127A = cst.tile([128, 8], F32); dqA = cst.tile([128, 8], F32)
            dqAg = cst.tile([128, 8], F32)
            q127O = cst.tile([128, 8], F32); dqOo = cst.tile([128, 8], F32)
            q127C = cst.tile([128, 8], F32); dqCg = cst.tile([128, 8], F32)

            dqrow_d = dr.tile([D], F32, tag="dqrow")
            xnew_d = dr.tile([TOK, D], F32, tag="xnew")
            ca_d = dr.tile([TOK, TOK], F32, tag="cad")

            # pools with managed lifetimes
            pW1c = tc.tile_pool(name="pW1", bufs=1, side="right")   # wi/wf/wg/Sb  [P0..g-end]
            pW1 = pW1c.__enter__()
            pB1c = tc.tile_pool(name="pB1", bufs=1)   # B_* rows     [P0..C-end]
            pb1 = pB1c.__enter__()
            pXAc = tc.tile_pool(name="pXA", bufs=2)   # xa + LN1 [..A-end]
            pXA = pXAc.__enter__()
            pLNc = tc.tile_pool(name="pLN", bufs=2)   # adaln scratch [..bcast]
            pLN = pLNc.__enter__()

            # ---------------- adaln (sharded) + AllGather ----------------
            adb_sb = pLN.tile([1, 6 * D], F32, tag="adb", bufs=1)
            nc.sync.dma_start(out=adb_sb, in_=adb_row[:, :])
            c_sb = pLN.tile([128, 8, B], F32, tag="csb")
            nc.sync.dma_start(out=c_sb, in_=c_cols[:, :, :])
            cs_sb = pLN.tile([128, 8, B], F32, tag="cssb")
            nc.scalar.activation(out=cs_sb, in_=c_sb, func=AF.Silu)

            psA = pmm("mmf")[:B, :]
            psB = pmm("mmi")[:B, 0:256]
            for j in range(8):
                adw_j = pLN.tile([128, 768], F32, tag="adw")
                nc.sync.dma_start(out=adw_j, in_=adw_sl[128 * j:128 * (j + 1), :])
                nc.tensor.matmul(psA, cs_sb[:, j, :], adw_j[:, 0:512],
                                 start=(j == 0), stop=(j == 7))
                nc.tensor.matmul(psB, cs_sb[:, j, :], adw_j[:, 512:768],
                                 start=(j == 0), stop=(j == 7))
            ad_sb = pLN.tile([B, 768], F32, tag="adsb")
            nc.scalar.copy(out=ad_sb[:, 0:512], in_=psA)
            nc.scalar.copy(out=ad_sb[:, 512:768], in_=psB)
            nc.sync.dma_start(out=cc1_in[:, :], in_=ad_sb)
            nc.gpsimd.collective_compute(
                "AllGather", AL.bypass, ins=[cc1_in[:]], outs=[cc1_out[:]],
                replica_groups=RG)

            # ------- overlap collective: x load + weights + LN1 stats ------
            xa = pXA.tile([128, 8, D], F32, tag="xa", bufs=1)
            nc.sync.dma_start(out=xa,
                              in_=x_sl[:, :].rearrange("(i p) d -> p i d", p=128))
            wg_sb = pW1.tile([128, 8, D], F8, tag="wg")
            nc.sync.dma_start(out=wg_sb,
                              in_=wgT[:, :].rearrange("(a p) q -> p a q", p=128))
            wi_all = pW1.tile([128, 8, 8, 128], F8, tag="wi")
            nc.sync.dma_start(
                out=wi_all,
                in_=wiT[:, :].rearrange("(a p) (b q) -> p a b q", p=128, q=128))
            wf_all = pW1.tile([128, 8, 8, 128], F8, tag="wf")
            nc.sync.dma_start(
                out=wf_all,
                in_=wfT[:, :].rearrange("(a p) (b q) -> p a b q", p=128, q=128))
            Sb_i = pW1.tile([128, D], F32, tag="sbi")
            Sb_f = pW1.tile([128, D], F32, tag="sbf")

            muA = pXA.tile([128, 8], F32, tag="muA", bufs=1)
            varA = pXA.tile([128, 8], F32, tag="varA", bufs=1)
            for i in range(8):
                st = pXA.tile([128, 2, 6], F32, tag="bst")
                xr = xa[:, i, :].rearrange("p (s d) -> p s d", s=2)
                for s2 in range(2):
                    nc.vector.bn_stats(out=st[:, s2, :], in_=xr[:, s2, :])
                mv = pXA.tile([128, 2], F32, tag="bmv")
                nc.vector.bn_aggr(out=mv, in_=st)
                nc.vector.tensor_copy(out=muA[:, i:i + 1], in_=mv[:, 0:1])
                nc.vector.tensor_copy(out=varA[:, i:i + 1], in_=mv[:, 1:2])
            rstdLN = _rsqrt(nc, pXA, varA, 1.0, 1e-6, [128, 8], "rLN")
            nmr = pXA.tile([128, 8], F32, tag="nmr", bufs=1)
            nc.vector.tensor_tensor(out=nmr, in0=muA, in1=rstdLN, op=AL.mult)
            nc.vector.tensor_scalar(out=nmr, in0=nmr, scalar1=-1.0,
                                    scalar2=None, op0=AL.mult)

            # ------- collect adaln params + broadcast rows ----------
            params_sb = pLN.tile([1, 6 * D], F32, tag="params", bufs=1)
            for r in range(8):
                ag_r = pLN.tile([B, 768], F32, tag="ag1")
                nc.sync.dma_start(out=ag_r, in_=cc1_out[4 * r:4 * (r + 1), :])
                pp1 = pmm("mmf")[:1, :]
                pp2 = pmm("mmi")[:1, 0:256]
                nc.tensor.matmul(pp1, bmask_sb, ag_r[:, 0:512], start=True, stop=True)
                nc.tensor.matmul(pp2, bmask_sb, ag_r[:, 512:768], start=True, stop=True)
                nc.scalar.copy(out=params_sb[:, 768 * r:768 * r + 512], in_=pp1)
                nc.scalar.copy(out=params_sb[:, 768 * r + 512:768 * (r + 1)], in_=pp2)
            nc.vector.tensor_tensor(out=params_sb, in0=params_sb, in1=adb_sb,
                                    op=AL.add)

            def bcast_row(pool, row_ap, bname, plus1=False):
                t = pool.tile([128, D], F32, tag=bname, name=bname)
                for ch in range(0, D, 512):
                    pb = pmm("mm")
                    nc.tensor.matmul(pb, ones_row, row_ap[:, ch:ch + 512],
                                     start=True, stop=True)
                    if plus1:
                        nc.scalar.activation(out=t[:, ch:ch + 512], in_=pb,
                                             func=AF.Identity, bias=1.0)
                    else:
                        nc.scalar.copy(out=t[:, ch:ch + 512], in_=pb)
                return t

            pr = params_sb.rearrange("one (six d) -> one six d", six=6)
            B_sh1 = bcast_row(pb1, pr[:, 0, :], "Bsh1")
            B_sc1 = bcast_row(pb1, pr[:, 1, :], "Bsc1", plus1=True)
            B_g1 = bcast_row(pb1, pr[:, 2, :], "Bg1")
            B_sh2 = bcast_row(pb1, pr[:, 3, :], "Bsh2")
            B_sc2 = bcast_row(pb1, pr[:, 4, :], "Bsc2", plus1=True)
            B_g2 = bcast_row(cst, pr[:, 5, :], "Bg2")
            B_gn = None if GN_ONES else bcast_row(pb1, gnr_sb, "Bgn")
            pLNc.__exit__(None, None, None)

            def quant_stats_sweep(src_get, n, amx, ssx, sb_pool, tagp):
                for i in range(n):
                    s = src_get(i)
                    nc.vector.tensor_reduce(out=amx[:, i:i + 1], in_=s, axis=AX.X,
                                            op=AL.max, apply_absolute_value=True)
                    scr = sb_pool.tile([128, s.free_size()], F32, bufs=1,
                                       tag=tagp + "sq", name=tagp + "sq")
                    nc.scalar.activation(out=scr, in_=s, func=AF.Square,
                                         accum_out=ssx[:, i:i + 1])

            def quant_batch(amx, ssx, dk, q127, dqt, dq_scaled, iws_scaled,
                            sb_pool, tagp):
                amc = sb_pool.tile([128, 8], F32, tag=tagp + "amc", name=tagp + "amc")
                nc.vector.tensor_scalar(out=amc, in0=amx, scalar1=1e-5,
                                        scalar2=None, op0=AL.max)
                rec = sb_pool.tile([128, 8], F32, tag=tagp + "rec", name=tagp + "rec")
                nc.vector.reciprocal(out=rec, in_=amc)
                nc.vector.tensor_scalar(out=q127, in0=rec, scalar1=127.0,
                                        scalar2=None, op0=AL.mult)
                rs = _rsqrt(nc, sb_pool, ssx, 1.0 / dk, 1e-8, [128, 8], tagp + "rs")
                nc.vector.tensor_tensor(out=dqt, in0=amc, in1=rs, op=AL.mult)
                nc.vector.tensor_scalar(out=dqt, in0=dqt, scalar1=1.0 / 127.0,
                                        scalar2=None, op0=AL.mult)
                if dq_scaled is not None:
                    nc.vector.tensor_scalar(out=dq_scaled, in0=dqt,
                                            scalar1=float(iws_scaled),
                                            scalar2=None, op0=AL.mult)

            cp_state = [0]

            def psum_copy(dst_ap, src_ap):
                k = cp_state[0] % 2
                cp_state[0] += 1
                if k == 0:
                    nc.scalar.copy(out=dst_ap, in_=src_ap)
                else:
                    nc.vector.tensor_copy(out=dst_ap, in_=src_ap)

            def quant_bf(src, q_col, dst, i, sb_pool, tagp):
                """src [128, D] f32 (tokens on partitions) -> int8-grid bf16,
                transposed to dst[:, j, 128i:...] (feature-major)."""
                t2 = sb_pool.tile([128, D], F32, bufs=2, tag=tagp + "t2",
                                  name=tagp + "t2")
                nc.scalar.activation(out=t2, in_=src, func=AF.Identity,
                                     scale=q_col, bias=cC[:, 0:1])
                kq = sb_pool.tile([128, D], BF, bufs=2, tag=tagp + "kq",
                                  name=tagp + "kq")
                nc.vector.tensor_scalar(out=kq, in0=t2, scalar1=C_MAGIC,
                                        scalar2=None, op0=AL.subtract)
                for g4 in range(0, 8, 4):
                    tph = ps.tile([128, 512], BF, tag="tpx", name="tpx", bufs=2)
                    for jj in range(4):
                        jb = g4 + jj
                        nc.tensor.transpose(tph[:, 128 * jj:128 * (jj + 1)],
                                            kq[:, 128 * jb:128 * (jb + 1)], identb)
                    psum_copy(dst[:, g4:g4 + 4, 128 * i:128 * (i + 1)],
                              tph[:].rearrange("p (a q) -> p a q", a=4))

            # ---------------- phase A: LN + modulate + quant ----------------
            pXQc = tc.tile_pool(name="pXQ", bufs=1, side="right")
            pXQ = pXQc.__enter__()
            xqT = pXQ.tile([128, 8, D], BF)
            with tc.tile_pool(name="pa", bufs=2) as pa:
                amA = pa.tile([128, 8], F32, tag="amA")
                ssA = pa.tile([128, 8], F32, tag="ssA")
                moda = pa.tile([128, 8, D], F32, tag="moda", bufs=1)
                for i in range(8):
                    u = pa.tile([128, D], F32, tag="u", bufs=2)
                    nc.scalar.activation(out=u, in_=xa[:, i, :], func=AF.Identity,
                                         scale=rstdLN[:, i:i + 1],
                                         bias=nmr[:, i:i + 1])
                    tt = pa.tile([128, D], F32, tag="tt", bufs=2)
                    nc.vector.tensor_tensor(out=tt, in0=u, in1=B_sc1, op=AL.mult)
                    nc.vector.tensor_tensor(out=moda[:, i, :], in0=tt, in1=B_sh1,
                                            op=AL.add)
                quant_stats_sweep(lambda i: moda[:, i, :], 8, amA, ssA, pa, "qa")
                quant_batch(amA, ssA, D, q127A, dqA, dqAg, iw["g"], pa, "qa")
                for i in range(8):
                    nc.sync.dma_start(
                        out=dqrow_d[128 * i:128 * (i + 1)].rearrange(
                            "(p one) -> p one", one=1),
                        in_=dqA[:, i:i + 1])
                dqrow_sb = pa.tile([1, D], F32, tag="dqrow")
                nc.sync.dma_start(out=dqrow_sb,
                                  in_=dqrow_d[:].rearrange("(one d) -> one d", one=1))
                oi = pa.tile([1, 128], F32, tag="oi")
                nc.vector.memset(oi, float(iw["i"]))
                of = pa.tile([1, 128], F32, tag="of")
                nc.vector.memset(of, float(iw["f"]))
                for ch in range(0, D, 512):
                    pb = pmm("mm")
                    nc.tensor.matmul(pb, oi, dqrow_sb[:, ch:ch + 512],
                                     start=True, stop=True)
                    nc.scalar.copy(out=Sb_i[:, ch:ch + 512], in_=pb)
                    pb2 = pmm("mm")
                    nc.tensor.matmul(pb2, of, dqrow_sb[:, ch:ch + 512],
                                     start=True, stop=True)
                    nc.vector.tensor_copy(out=Sb_f[:, ch:ch + 512], in_=pb2)
                for i in range(8):
                    quant_bf(moda[:, i, :], q127A[:, i:i + 1], xqT, i, pa, "ra")
            pXAc.__exit__(None, None, None)

            # ---------------- phase B: i/f matmuls + scan ----------------
            pSGc = tc.tile_pool(name="pSG", bufs=1)   # gs + hT [B..o-end]
            pSG = pSGc.__enter__()
            gs = pSG.tile([128, 8, D], F32, tag="gs")
            hT = pSG.tile([128, 8, D], F32, tag="hT")
            pHAc = tc.tile_pool(name="pHA", bufs=1)   # ha [scan..fixup]
            pHA = pHAc.__enter__()
            ha = pHA.tile([128, 8, TOK], F32)
            pbsc = tc.tile_pool(name="pbs", bufs=1)   # scan scratch
            pb = pbsc.__enter__()
            for m in range(8):
                ft = pb.tile([128, TOK], F32, tag="ftm")
                it = pb.tile([128, TOK], F32, tag="itm")
                for cki, ck in enumerate(range(0, TOK, 512)):
                    pf = pmm("mmf")
                    pi = pmm("mmi")
                    for j in range(8):
                        nc.tensor.matmul(pf, wf_all[:, j, m, :],
                                         xqT[:, j, ck:ck + 512],
                                         start=(j == 0), stop=(j == 7))
                    for j in range(8):
                        nc.tensor.matmul(pi, wi_all[:, j, m, :],
                                         xqT[:, j, ck:ck + 512],
                                         start=(j == 0), stop=(j == 7))
                    nc.vector.tensor_tensor(out=ft[:, ck:ck + 512], in0=pf,
                                            in1=Sb_f[:, ck:ck + 512], op=AL.mult)
                    nc.vector.tensor_tensor(out=it[:, ck:ck + 512], in0=pi,
                                            in1=Sb_i[:, ck:ck + 512], op=AL.mult)
                sigf = pb.tile([128, TOK], F32, tag="sigf")
                nc.scalar.activation(out=sigf, in_=ft, func=AF.Sigmoid)
                sili = pb.tile([128, TOK], F32, tag="sili")
                nc.scalar.activation(out=sili, in_=it, func=AF.Silu)
                omf = pb.tile([128, TOK], F32, tag="omf")
                nc.vector.tensor_scalar(out=omf, in0=sigf, scalar1=-1.0,
                                        scalar2=1.0, op0=AL.mult, op1=AL.add)
                ifin = pb.tile([128, TOK], F32, tag="ifin")
                nc.vector.tensor_tensor(out=ifin, in0=sili, in1=omf, op=AL.mult)
                nc.vector.tensor_tensor_scan(ha[:, m, :], sigf, ifin, 0.0,
                                             op0=AL.mult, op1=AL.add)
                cam = pb.tile([128, TOK], F32, tag="cam", bufs=1)
                nc.vector.tensor_tensor_scan(cam, sigf, sigf, 1.0,
                                             op0=AL.mult, op1=AL.bypass)
                nc.sync.dma_start(out=ca_d[128 * m:128 * (m + 1), :], in_=cam)
            nc.sync.dma_start(
                out=cc2_in[:].rearrange("(a p) -> p a", p=128),
                in_=ha[:, :, TOK - 1:TOK].rearrange("p a one -> p (a one)"))
            nc.gpsimd.collective_compute(
                "AllGather", AL.bypass, ins=[cc2_in[:]], outs=[cc2_out[:]],
                replica_groups=RG)
            pbsc.__exit__(None, None, None)

            # ---- overlap the collective: g = silu(xq @ wg) * gnorm ----
            pb2c = tc.tile_pool(name="pb2", bufs=1)   # g + fixup scratch
            pb2 = pb2c.__enter__()
            for t in range(8):
                for ck in range(0, D, 512):
                    pg = pmm("mm")
                    for j in range(8):
                        nc.tensor.matmul(pg, xqT[:, j,
                                                 128 * t:128 * (t + 1)],
                                         wg_sb[:, j, ck:ck + 512],
                                         start=(j == 0), stop=(j == 7))
                    if GN_ONES:
                        nc.scalar.activation(out=gs[:, t, ck:ck + 512], in_=pg,
                                             func=AF.Silu, scale=dqAg[:, t:t + 1])
                    else:
                        scr = pb2.tile([128, 512], F32, tag="gscr", bufs=2)
                        nc.scalar.activation(out=scr, in_=pg, func=AF.Silu,
                                             scale=dqAg[:, t:t + 1])
                        nc.vector.tensor_tensor(out=gs[:, t, ck:ck + 512], in0=scr,
                                                in1=B_gn[:, ck:ck + 512], op=AL.mult)
            pXQc.__exit__(None, None, None)
            pW1c.__exit__(None, None, None)

            # ---- carry fixup + transpose h -> hT ----
            ag2 = pb2.tile([N_CORES, D], F32, tag="ag2")
            nc.sync.dma_start(out=ag2, in_=cc2_out[:, :])
            for m in range(8):
                pc0 = pmm("mm")[:, 0:1]
                nc.tensor.matmul(pc0, ag2[:, 128 * m:128 * (m + 1)], mask_sb,
                                 start=True, stop=True)
                carry = pb2.tile([128, 1], F32, tag="carry", bufs=2)
                nc.scalar.copy(out=carry, in_=pc0)
                cam2 = pb2.tile([128, TOK], F32, tag="cam2", bufs=2)
                nc.sync.dma_start(out=cam2, in_=ca_d[128 * m:128 * (m + 1), :])
                hfix = pb2.tile([128, TOK], F32, tag="hfix", bufs=2)
                nc.vector.scalar_tensor_tensor(out=hfix, in0=cam2,
                                               scalar=carry, in1=ha[:, m, :],
                                               op0=AL.mult, op1=AL.add)
                for g4 in range(0, 8, 4):
                    tp = pmm("mm")
                    for jj in range(4):
                        t_i = g4 + jj
                        nc.tensor.transpose(tp[:, 128 * jj:128 * (jj + 1)],
                                            hfix[:, 128 * t_i:128 * (t_i + 1)],
                                            identf)
                    psum_copy(hT[:, g4:g4 + 4, 128 * m:128 * (m + 1)],
                              tp[:].rearrange("p (a q) -> p a q", a=4))
            pb2c.__exit__(None, None, None)
            pHAc.__exit__(None, None, None)

            # ---------------- o-stage ----------------
            pX2c = tc.tile_pool(name="pX2", bufs=1, side="right")
            x2qT = pX2c.__enter__().tile([128, 8, D], BF)
            pOQc = tc.tile_pool(name="pOQ", bufs=1, side="right")
            pOQ = pOQc.__enter__()
            oqT = pOQ.tile([128, 8, D], BF)
            with tc.tile_pool(name="po", bufs=2) as po:
                mshA = po.tile([128, 8, 16], F32, tag="msh", bufs=1)
                for t in range(8):
                    sq = po.tile([128, D], F32, tag="sq", bufs=2)
                    nc.vector.tensor_tensor(out=sq, in0=hT[:, t, :],
                                            in1=hT[:, t, :], op=AL.mult)
                    nc.vector.tensor_reduce(
                        out=mshA[:, t, :],
                        in_=sq.rearrange("p (h d) -> p h d", h=NH),
                        axis=AX.X, op=AL.add)
                rstdH = _rsqrt(nc, po,
                               mshA[:, :, :].rearrange("p a b -> p (a b)"),
                               1.0 / HD, 1e-5, [128, 128], "rH")
                rH = rstdH.rearrange("p (a b) -> p a b", a=8)
                amO = po.tile([128, 8], F32, tag="amO")
                ssO = po.tile([128, 8], F32, tag="ssO")
                for t in range(8):
                    hn = po.tile([128, D], F32, tag="hn", bufs=2)
                    rb = bass.AP(tensor=rH.tensor, offset=rH[:, t, :].offset,
                                 ap=[rH.ap[0], [1, NH], [0, HD]])
                    nc.vector.tensor_tensor(
                        out=hn.rearrange("p (h d) -> p h d", h=NH),
                        in0=hT[:, t, :].rearrange("p (h d) -> p h d", h=NH),
                        in1=rb, op=AL.mult)
                    # oa overwrites gs in place
                    nc.vector.tensor_tensor(out=gs[:, t, :], in0=hn,
                                            in1=gs[:, t, :], op=AL.mult)
                quant_stats_sweep(lambda t: gs[:, t, :], 8, amO, ssO, po, "qo")
                quant_batch(amO, ssO, D, q127O, dqOo, dqOo, 1.0, po, "qo")
                nc.vector.tensor_scalar(out=dqOo, in0=dqOo, scalar1=float(iw["o"]),
                                        scalar2=None, op0=AL.mult)
                for t in range(8):
                    quant_bf(gs[:, t, :], q127O[:, t:t + 1], oqT, t, po, "ro")
            pSGc.__exit__(None, None, None)

            # ---------------- phase C: wo matmul + residual + LN2 ----------
            pGWc = tc.tile_pool(name="pGW", bufs=1, side="right")
            pDWc = tc.tile_pool(name="pDW", bufs=1, side="right")
            pM2c = tc.tile_pool(name="pM2", bufs=1)
            mod2 = pM2c.__enter__().tile([128, 8, D], F32)
            with tc.tile_pool(name="pc", bufs=2) as pc:
                wo_sb = pc.tile([128, 8, D], F8, tag="wosb", bufs=1)
                nc.sync.dma_start(out=wo_sb,
                                  in_=woT[:, :].rearrange("(a p) q -> p a q", p=128))
                muC = pc.tile([128, 8], F32, tag="muC")
                varC = pc.tile([128, 8], F32, tag="varC")
                for t in range(8):
                    xa2 = pc.tile([128, D], F32, tag="xa2", bufs=2)
                    nc.sync.dma_start(out=xa2, in_=x_sl[128 * t:128 * (t + 1), :])
                    xn = pc.tile([128, D], F32, tag="xn", bufs=2)
                    for ck in range(0, D, 512):
                        pw = pmm("mmf")
                        for j in range(8):
                            nc.tensor.matmul(pw, oqT[:, j,
                                                     128 * t:128 * (t + 1)],
                                             wo_sb[:, j, ck:ck + 512],
                                             start=(j == 0), stop=(j == 7))
                        at = pc.tile([128, 512], F32, tag="at", bufs=2)
                        nc.vector.tensor_scalar(out=at, in0=pw,
                                                scalar1=dqOo[:, t:t + 1],
                                                scalar2=None, op0=AL.mult)
                        ug = pc.tile([128, 512], F32, tag="ug", bufs=2)
                        nc.vector.tensor_tensor(out=ug, in0=at,
                                                in1=B_g1[:, ck:ck + 512], op=AL.mult)
                        nc.vector.tensor_tensor(out=xn[:, ck:ck + 512], in0=ug,
                                                in1=xa2[:, ck:ck + 512], op=AL.add)
                    nc.sync.dma_start(out=xnew_d[128 * t:128 * (t + 1), :], in_=xn)
                    st = pc.tile([128, 2, 6], F32, tag="bst2")
                    xrr = xn.rearrange("p (s d) -> p s d", s=2)
                    for s2 in range(2):
                        nc.vector.bn_stats(out=st[:, s2, :], in_=xrr[:, s2, :])
                    mv = pc.tile([128, 2], F32, tag="bmv2")
                    nc.vector.bn_aggr(out=mv, in_=st)
                    nc.vector.tensor_copy(out=muC[:, t:t + 1], in_=mv[:, 0:1])
                    nc.vector.tensor_copy(out=varC[:, t:t + 1], in_=mv[:, 1:2])
                # wo matmuls done -> free oqT, start gw load
                pOQc.__exit__(None, None, None)
                gw_sb = pGWc.__enter__().tile([128, 8, 2 * MLP], F8)
                nc.sync.dma_start(
                    out=gw_sb,
                    in_=gwT[:, :].rearrange("(a p) q -> p a q", p=128))
                rstdC = _rsqrt(nc, pc, varC, 1.0, 1e-6, [128, 8], "rC")
                nmrC = pc.tile([128, 8], F32, tag="nmrC")
                nc.vector.tensor_tensor(out=nmrC, in0=muC, in1=rstdC, op=AL.mult)
                nc.vector.tensor_scalar(out=nmrC, in0=nmrC, scalar1=-1.0,
                                        scalar2=None, op0=AL.mult)
                for t in range(8):
                    xn2 = pc.tile([128, D], F32, tag="xn2", bufs=2)
                    nc.sync.dma_start(out=xn2, in_=xnew_d[128 * t:128 * (t + 1), :])
                    u2 = pc.tile([128, D], F32, tag="u2", bufs=1)
                    nc.scalar.activation(out=u2, in_=xn2, func=AF.Identity,
                                         scale=rstdC[:, t:t + 1],
                                         bias=nmrC[:, t:t + 1])
                    tt2 = pc.tile([128, D], F32, tag="tt2", bufs=1)
                    nc.vector.tensor_tensor(out=tt2, in0=u2, in1=B_sc2, op=AL.mult)
                    nc.vector.tensor_tensor(out=mod2[:, t, :], in0=tt2, in1=B_sh2,
                                            op=AL.add)
            with tc.tile_pool(name="pcq", bufs=2) as pcq:
                amC = pcq.tile([128, 8], F32, tag="amC")
                ssC = pcq.tile([128, 8], F32, tag="ssC")
                quant_stats_sweep(lambda t: mod2[:, t, :], 8, amC, ssC, pcq, "qc")
                quant_batch(amC, ssC, D, q127C, dqCg, dqCg, iw["gate"], pcq, "qc")
                for t in range(8):
                    quant_bf(mod2[:, t, :], q127C[:, t:t + 1], x2qT, t, pcq, "rc")
            pM2c.__exit__(None, None, None)
            pB1c.__exit__(None, None, None)

            # ---------------- phase D: MLP (SBUF-resident, pipelined) ------
            dw_sb = pDWc.__enter__().tile([128, 32, D], F8)
            nc.sync.dma_start(out=dw_sb,
                              in_=dwT[:, :].rearrange("(a p) q -> p a q", p=128))
            with tc.tile_pool(name="pd", bufs=2) as pd:
                def emit_gate_half(t, h2_h, amD_t, ssD_t, half):
                    for ck in range(4):
                        c0 = 2048 * half + 512 * ck
                        pg = pmm("mmf")
                        py = pmm("mmi")
                        for j in range(8):
                            nc.tensor.matmul(pg, x2qT[:, j,
                                                      128 * t:128 * (t + 1)],
                                             gw_sb[:, j, c0:c0 + 512],
                                             start=(j == 0), stop=(j == 7))
                        for j in range(8):
                            nc.tensor.matmul(py, x2qT[:, j,
                                                      128 * t:128 * (t + 1)],
                                             gw_sb[:, j,
                                                   MLP + c0:MLP + c0 + 512],
                                             start=(j == 0), stop=(j == 7))
                        sil = pd.tile([128, 512], F32, tag="sil", bufs=1)
                        nc.scalar.activation(out=sil, in_=pg, func=AF.Silu,
                                             scale=dqCg[:, t:t + 1])
                        hc = 512 * ck
                        nc.vector.scalar_tensor_tensor(
                            out=h2_h[:, hc:hc + 512], in0=py,
                            scalar=dqCg[:, t:t + 1], in1=sil,
                            op0=AL.mult, op1=AL.mult)
                        cki = 4 * half + ck
                        nc.vector.tensor_reduce(out=amD_t[:, cki:cki + 1],
                                                in_=h2_h[:, hc:hc + 512],
                                                axis=AX.X, op=AL.max,
                                                apply_absolute_value=True)
                        scr = pd.tile([128, 512], F32, tag="sqd", bufs=1)
                        nc.scalar.activation(out=scr, in_=h2_h[:, hc:hc + 512],
                                             func=AF.Square,
                                             accum_out=ssD_t[:, cki:cki + 1])

                def emit_quant_h2(t, h2_hs, amD_t, ssD_t, h2q_t):
                    amD = pd.tile([128, 1], F32, tag="amD", bufs=2)
                    nc.vector.tensor_reduce(out=amD, in_=amD_t, axis=AX.X,
                                            op=AL.max)
                    ssD = pd.tile([128, 1], F32, tag="ssD", bufs=2)
                    nc.vector.tensor_reduce(out=ssD, in_=ssD_t, axis=AX.X,
                                            op=AL.add)
                    amc = pd.tile([128, 1], F32, tag="qdamc", bufs=2)
                    nc.vector.tensor_scalar(out=amc, in0=amD, scalar1=1e-5,
                                            scalar2=None, op0=AL.max)
                    rec = pd.tile([128, 1], F32, tag="qdrec", bufs=2)
                    nc.vector.reciprocal(out=rec, in_=amc)
                    q127 = pd.tile([128, 1], F32, tag="qdq", bufs=2)
                    nc.vector.tensor_scalar(out=q127, in0=rec, scalar1=127.0,
                                            scalar2=None, op0=AL.mult)
                    rs = _rsqrt(nc, pd, ssD, 1.0 / MLP, 1e-8, [128, 1], "rD")
                    dq = pd.tile([128, 1], F32, tag="qddq", bufs=2)
                    nc.vector.tensor_tensor(out=dq, in0=amc, in1=rs, op=AL.mult)
                    nc.vector.tensor_scalar(out=dq, in0=dq,
                                            scalar1=float(iw["down"]) / 127.0,
                                            scalar2=None, op0=AL.mult)
                    for half in range(2):
                        h2_h = h2_hs[half]
                        t2d = pd.tile([128, 2048], F32, tag="t2d", bufs=1)
                        nc.scalar.activation(out=t2d, in_=h2_h, func=AF.Identity,
                                             scale=q127[:, 0:1], bias=cC[:, 0:1])
                        kq = pd.tile([128, 2048], BF, tag="kq", bufs=1)
                        nc.vector.tensor_scalar(out=kq, in0=t2d, scalar1=C_MAGIC,
                                                scalar2=None, op0=AL.subtract)
                        for g8 in range(0, 16, 4):
                            tp = ps.tile([128, 512], BF, tag="tpx", name="tpx",
                                         bufs=2)
                            for jj in range(4):
                                j2 = g8 + jj
                                nc.tensor.transpose(
                                    tp[:, 128 * jj:128 * (jj + 1)],
                                    kq[:, 128 * j2:128 * (j2 + 1)], identb)
                            psum_copy(h2q_t[:, 16 * half + g8:16 * half + g8 + 4, :],
                                      tp[:].rearrange("p (a q) -> p a q", a=4))
                    return dq

                def emit_down(t, h2q_t, dq):
                    xn3 = pd.tile([128, D], F32, tag="xn3", bufs=1)
                    nc.sync.dma_start(out=xn3,
                                      in_=xnew_d[128 * t:128 * (t + 1), :])
                    outt = pd.tile([128, D], F32, tag="outt", bufs=2)
                    for ck in range(0, D, 512):
                        pdn = pmm("mm")
                        for j2 in range(32):
                            nc.tensor.matmul(pdn, h2q_t[:, j2, :],
                                             dw_sb[:, j2, ck:ck + 512],
                                             start=(j2 == 0), stop=(j2 == 31))
                        u2 = pd.tile([128, 512], F32, tag="u2d", bufs=2)
                        nc.vector.tensor_scalar(out=u2, in0=pdn, scalar1=dq,
                                                scalar2=None, op0=AL.mult)
                        v2 = pd.tile([128, 512], F32, tag="v2d", bufs=2)
                        nc.vector.tensor_tensor(out=v2, in0=u2,
                                                in1=B_g2[:, ck:ck + 512], op=AL.mult)
                        nc.vector.tensor_tensor(out=outt[:, ck:ck + 512], in0=v2,
                                                in1=xn3[:, ck:ck + 512], op=AL.add)
                    nc.sync.dma_start(out=out_sl[128 * t:128 * (t + 1), :], in_=outt)

                prev = None
                for t in range(8):
                    h2_a = pd.tile([128, 2048], F32, tag="h2h", bufs=4)
                    h2_b = pd.tile([128, 2048], F32, tag="h2h", bufs=4)
                    amD_t = pd.tile([128, 8], F32, tag="amDt", bufs=2)
                    ssD_t = pd.tile([128, 8], F32, tag="ssDt", bufs=2)
                    emit_gate_half(t, h2_a, amD_t, ssD_t, 0)
                    emit_gate_half(t, h2_b, amD_t, ssD_t, 1)
                    if prev is not None:
                        pt, ph2s, pam, pss = prev
                        h2q_t = pd.tile([128, 32, 128], BF, tag="h2qt", bufs=2)
                        dq = emit_quant_h2(pt, ph2s, pam, pss, h2q_t)
                        emit_down(pt, h2q_t, dq)
                    prev = (t, (h2_a, h2_b), amD_t, ssD_t)
                pt, ph2s, pam, pss = prev
                h2q_t = pd.tile([128, 32, 128], BF, tag="h2qt", bufs=2)
                dq = emit_quant_h2(pt, ph2s, pam, pss, h2q_t)
                emit_down(pt, h2q_t, dq)
            pDWc.__exit__(None, None, None)
            pGWc.__exit__(None, None, None)
            pX2c.__exit__(None, None, None)

    nc.finalize()
    return nc


@functools.lru_cache(maxsize=2)
def _build_cached(iw_items, gn_ones):
    return _build(dict(iw_items), gn_ones)


def kernel(x, c, adaln_w, adaln_b, wi, wf, wg, gnorm_w, wo, gate_w, down_w):
    x = np.ascontiguousarray(np.asarray(x, dtype=np.float32))
    c = np.ascontiguousarray(np.asarray(c, dtype=np.float32))
    adaln_w = np.asarray(adaln_w, dtype=np.float32)
    adaln_b = np.asarray(adaln_b, dtype=np.float32)
    gnorm_w = np.asarray(gnorm_w, dtype=np.float32)

    mi, iwi = _quant_w(np.asarray(wi, dtype=np.float32))
    mf, iwf = _quant_w(np.asarray(wf, dtype=np.float32))
    mg, iwg = _quant_w(np.asarray(wg, dtype=np.float32))
    mo, iwo = _quant_w(np.asarray(wo, dtype=np.float32))
    mgate, iwgate = _quant_w(np.asarray(gate_w, dtype=np.float32))
    mdown, iwdown = _quant_w(np.asarray(down_w, dtype=np.float32))

    iw = {"i": float(iwi), "f": float(iwf), "g": float(iwg), "o": float(iwo),
          "gate": float(iwgate), "down": float(iwdown)}
    gn_ones = bool(np.allclose(gnorm_w, 1.0))
    nc = _build_cached(tuple(sorted(iw.items())), gn_ones)

    wiT_h = np.ascontiguousarray(mi.T)
    wfT_h = np.ascontiguousarray(mf.T)
    wgT_h = np.ascontiguousarray(mg.T)
    woT_h = np.ascontiguousarray(mo.T)
    gwT_h = np.ascontiguousarray(mgate.T)
    dwT_h = np.ascontiguousarray(mdown.T)
    adwT = np.ascontiguousarray(adaln_w.T)          # [D, 6D]
    adb_row_h = np.ascontiguousarray(adaln_b[None, :])
    gnr_h = np.ascontiguousarray(np.tile(gnorm_w, NH)[None, :])
    c_cols_h = np.ascontiguousarray(
        c.T.reshape(8, 128, B).transpose(1, 0, 2))   # [128, 8, B]

    in_maps = []
    for core in range(N_CORES):
        b, half = core // 2, core % 2
        mask = np.zeros((N_CORES, 1), np.float32)
        if half == 1:
            mask[core - 1, 0] = 1.0
        bm = np.zeros((B, 1), np.float32)
        bm[b, 0] = 1.0
        in_maps.append({
            "x_sl": np.ascontiguousarray(x[b, half * TOK:(half + 1) * TOK, :]),
            "c_cols": c_cols_h,
            "adw_sl": np.ascontiguousarray(adwT[:, 768 * core:768 * (core + 1)]),
            "adb_row": adb_row_h,
            "mask8": mask,
            "bmask": bm,
            "gnr": gnr_h,
            "wiT": wiT_h, "wfT": wfT_h, "wgT": wgT_h, "woT": woT_h,
            "gwT": gwT_h, "dwT": dwT_h,
        })

    res = run_bass_kernel_spmd(nc, in_maps, core_ids=list(range(N_CORES)))
    out = np.zeros((B, T, D), np.float32)
    for core in range(N_CORES):
        b, half = core // 2, core % 2
        out[b, half * TOK:(half + 1) * TOK, :] = res.results[core]["out_sl"]
    return out


# revision 25
# speedup vs baseline: 1.8823x; 1.0269x over previous
"""Trainium2 Bass kernel for nn_DiTBlock (HGRN-attention DiT block).

Sharding: 8 cores = 4 batches x 2 half-sequences (1024 tokens each).
All big matmuls run as EXACT integer arithmetic: activations are
int8-grid quantized to bf16 (matching the reference bitlinear rounding
bit-for-bit), ternary weights are stored as fp8 (halving weight SBUF
and HBM traffic); mixed bf16 x fp8 matmuls run at full bf16 PE rate.
The MLP is fully SBUF-resident (gate/down weights live on-chip, h2
rows are produced, quantized, and consumed per 128-token block with no
DRAM round trip). The carry AllGather is hidden behind the wg matmul;
elementwise work is split across DVE and Act engines (the Pool engine
is ~20x too slow for bulk elementwise and is used only for
collectives). The time-recurrence h_t = f_t*h_{t-1} + i_t uses the DVE
tensor_tensor_scan; the half-sequence boundary carry crosses cores via
AllGather + one-hot mask matmul. adaln params are computed on-device,
sharded 8 ways over the 6144 outputs and AllGathered.
"""
import functools
import numpy as np
import ml_dtypes

import concourse.bass as bass
import concourse.bacc as bacc_mod
import concourse.mybir as mybir
import concourse.tile as tile
from concourse.masks import make_identity
from concourse.bass_utils import run_bass_kernel_spmd

E4M3 = ml_dtypes.float8_e4m3
F32 = mybir.dt.float32
BF = mybir.dt.bfloat16
F8 = mybir.dt.float8e4
U32 = mybir.dt.uint32
AL = mybir.AluOpType
AF = mybir.ActivationFunctionType
AX = mybir.AxisListType
PM = mybir.MatmulPerfMode

B, T, D = 4, 2048, 1024
TOK = 1024          # tokens per core
NH, HD = 16, 64
MLP = 4096
N_CORES = 8
C_MAGIC = float(1.5 * 2 ** 23)
C16 = 16.0 * C_MAGIC
MAGIC_U32 = 0x5F3759DF


def _quant_w(w):
    invws = float(np.clip(np.abs(w).mean(dtype=np.float64), 1e-5, None))
    m = np.clip(np.round(w.astype(np.float64) / invws), -1, 1).astype(np.float32)
    return np.ascontiguousarray(m.astype(E4M3)), np.float32(invws)


def _rsqrt(nc, sb, x_ap, scale, bias, shape, tag):
    """out = rsqrt(x*scale + bias), Newton on DVE. Returns a new tile."""
    t = sb.tile(shape, F32, tag=tag + "_t", name=tag + "_t")
    nc.vector.tensor_scalar(out=t, in0=x_ap, scalar1=float(scale),
                            scalar2=float(bias), op0=AL.mult, op1=AL.add)
    y = sb.tile(shape, F32, tag=tag + "_y", name=tag + "_y")
    sh = sb.tile(shape, F32, tag=tag + "_s", name=tag + "_s")
    nc.vector.tensor_scalar(out=sh[:].bitcast(U32), in0=t[:].bitcast(U32),
                            scalar1=1, scalar2=None, op0=AL.logical_shift_right)
    mg = sb.tile(shape, F32, tag=tag + "_m", name=tag + "_m")
    nc.vector.memset(mg[:].bitcast(U32), MAGIC_U32)
    nc.vector.tensor_tensor(out=y[:].bitcast(U32), in0=mg[:].bitcast(U32),
                            in1=sh[:].bitcast(U32), op=AL.subtract)
    e = sb.tile(shape, F32, tag=tag + "_e", name=tag + "_e")
    for _ in range(3):
        nc.vector.tensor_tensor(out=e, in0=y, in1=y, op=AL.mult)
        nc.vector.tensor_tensor(out=e, in0=e, in1=t, op=AL.mult)
        nc.vector.tensor_scalar(out=e, in0=e, scalar1=-0.5, scalar2=1.5,
                                op0=AL.mult, op1=AL.add)
        nc.vector.tensor_tensor(out=y, in0=y, in1=e, op=AL.mult)
    return y


def _wpair(ap2):
    """[128, X] weight AP -> [128, 2, X] with stride-0 pair dim."""
    return bass.AP(tensor=ap2.tensor, offset=ap2.offset,
                   ap=[ap2.ap[0], [0, 2], ap2.ap[-1]])


def _build(iw, gn_ones=True):
    GN_ONES = gn_ones
    nc = bacc_mod.Bacc("TRN2", target_bir_lowering=False)

    x_sl = nc.declare_dram_parameter("x_sl", [TOK, D], F32, isOutput=False)
    c_cols = nc.declare_dram_parameter("c_cols", [128, 8, B], F32, isOutput=False)
    adw_sl = nc.declare_dram_parameter("adw_sl", [D, 768], F32, isOutput=False)
    adb_row = nc.declare_dram_parameter("adb_row", [1, 6 * D], F32, isOutput=False)
    mask8 = nc.declare_dram_parameter("mask8", [N_CORES, 1], F32, isOutput=False)
    bmask = nc.declare_dram_parameter("bmask", [B, 1], F32, isOutput=False)
    gnr = nc.declare_dram_parameter("gnr", [1, D], F32, isOutput=False)
    wiT = nc.declare_dram_parameter("wiT", [D, D], F8, isOutput=False)
    wfT = nc.declare_dram_parameter("wfT", [D, D], F8, isOutput=False)
    wgT = nc.declare_dram_parameter("wgT", [D, D], F8, isOutput=False)
    woT = nc.declare_dram_parameter("woT", [D, D], F8, isOutput=False)
    gwT = nc.declare_dram_parameter("gwT", [D, 2 * MLP], F8, isOutput=False)
    dwT = nc.declare_dram_parameter("dwT", [MLP, D], F8, isOutput=False)
    out_sl = nc.declare_dram_parameter("out_sl", [TOK, D], F32, isOutput=True)

    cc1_in = nc.dram_tensor("cc1_in", [B, 768], F32)
    cc1_out = nc.dram_tensor("cc1_out", [N_CORES * B, 768], F32, addr_space="Shared")
    cc2_in = nc.dram_tensor("cc2_in", [D], F32)
    cc2_out = nc.dram_tensor("cc2_out", [N_CORES, D], F32, addr_space="Shared")

    RG = [list(range(N_CORES))]

    with tile.TileContext(nc) as tc:
        import contextlib
        es = contextlib.ExitStack()
        with es:
            cst = es.enter_context(tc.tile_pool(name="cst", bufs=1))
            ps = es.enter_context(tc.tile_pool(name="ps", bufs=1, space="PSUM"))
            dr = es.enter_context(tc.tile_pool(name="dr", bufs=1, space="DRAM"))

            def pmm(tag="mm", bufs=2):
                return ps.tile([128, 512], F32, tag=tag, name=tag, bufs=bufs)

            # ---------------- consts ----------------
            identb = cst.tile([128, 128], BF)
            make_identity(nc, identb)
            identf = cst.tile([128, 128], F32)
            make_identity(nc, identf)
            ident8 = cst.tile([128, 128], F8)
            make_identity(nc, ident8)
            ones_row = cst.tile([1, 128], F32)
            nc.vector.memset(ones_row, 1.0)
            cC = cst.tile([128, 1], F32)
            nc.vector.memset(cC, C_MAGIC)
            mask_sb = cst.tile([N_CORES, 1], F32)
            nc.sync.dma_start(out=mask_sb, in_=mask8[:, :])
            bmask_sb = cst.tile([B, 1], F32)
            nc.sync.dma_start(out=bmask_sb, in_=bmask[:, :])
            gnr_sb = cst.tile([1, D], F32)
            nc.sync.dma_start(out=gnr_sb, in_=gnr[:, :])

            # long-lived small stat tiles
            q127A = cst.tile([128, 8], F32); dqA = cst.tile([128, 8], F32)
            dqAg = cst.tile([128, 8], F32)
            q127O = cst.tile([128, 8], F32); dqOo = cst.tile([128, 8], F32)
            q127C = cst.tile([128, 8], F32); dqCg = cst.tile([128, 8], F32)

            dqrow_d = dr.tile([D], F32, tag="dqrow")
            xnew_d = dr.tile([TOK, D], F32, tag="xnew")
            ca_d = dr.tile([TOK, TOK], F32, tag="cad")

            # pools with managed lifetimes
            pW1c = tc.tile_pool(name="pW1", bufs=1, side="right")   # wi/wf/wg/Sb  [P0..g-end]
            pW1 = pW1c.__enter__()
            pB1c = tc.tile_pool(name="pB1", bufs=1)   # B_* rows     [P0..C-end]
            pb1 = pB1c.__enter__()
            pXAc = tc.tile_pool(name="pXA", bufs=2)   # xa + LN1 [..A-end]
            pXA = pXAc.__enter__()
            pLNc = tc.tile_pool(name="pLN", bufs=2)   # adaln scratch [..bcast]
            pLN = pLNc.__enter__()

            # ---------------- adaln (sharded) + AllGather ----------------
            adb_sb = pLN.tile([1, 6 * D], F32, tag="adb", bufs=1)
            nc.sync.dma_start(out=adb_sb, in_=adb_row[:, :])
            c_sb = pLN.tile([128, 8, B], F32, tag="csb")
            nc.sync.dma_start(out=c_sb, in_=c_cols[:, :, :])
            cs_sb = pLN.tile([128, 8, B], F32, tag="cssb")
            nc.scalar.activation(out=cs_sb, in_=c_sb, func=AF.Silu)

            psA = pmm("mmf")[:B, :]
            psB = pmm("mmi")[:B, 0:256]
            for j in range(8):
                adw_j = pLN.tile([128, 768], F32, tag="adw")
                nc.sync.dma_start(out=adw_j, in_=adw_sl[128 * j:128 * (j + 1), :])
                nc.tensor.matmul(psA, cs_sb[:, j, :], adw_j[:, 0:512],
                                 start=(j == 0), stop=(j == 7))
                nc.tensor.matmul(psB, cs_sb[:, j, :], adw_j[:, 512:768],
                                 start=(j == 0), stop=(j == 7))
            ad_sb = pLN.tile([B, 768], F32, tag="adsb")
            nc.scalar.copy(out=ad_sb[:, 0:512], in_=psA)
            nc.scalar.copy(out=ad_sb[:, 512:768], in_=psB)
            nc.sync.dma_start(out=cc1_in[:, :], in_=ad_sb)
            nc.gpsimd.collective_compute(
                "AllGather", AL.bypass, ins=[cc1_in[:]], outs=[cc1_out[:]],
                replica_groups=RG)

            # ------- overlap collective: x load + weights + LN1 stats ------
            xa = pXA.tile([128, 8, D], F32, tag="xa", bufs=1)
            nc.sync.dma_start(out=xa,
                              in_=x_sl[:, :].rearrange("(i p) d -> p i d", p=128))
            wg_sb = pW1.tile([128, 8, D], F8, tag="wg")
            nc.sync.dma_start(out=wg_sb,
                              in_=wgT[:, :].rearrange("(a p) q -> p a q", p=128))
            wi_all = pW1.tile([128, 8, 8, 128], F8, tag="wi")
            nc.sync.dma_start(
                out=wi_all,
                in_=wiT[:, :].rearrange("(a p) (b q) -> p a b q", p=128, q=128))
            wf_all = pW1.tile([128, 8, 8, 128], F8, tag="wf")
            nc.sync.dma_start(
                out=wf_all,
                in_=wfT[:, :].rearrange("(a p) (b q) -> p a b q", p=128, q=128))
            Sb_i = pW1.tile([128, D], F32, tag="sbi")
            Sb_f = pW1.tile([128, D], F32, tag="sbf")

            muA = pXA.tile([128, 8], F32, tag="muA", bufs=1)
            varA = pXA.tile([128, 8], F32, tag="varA", bufs=1)
            for i in range(8):
                st = pXA.tile([128, 2, 6], F32, tag="bst")
                xr = xa[:, i, :].rearrange("p (s d) -> p s d", s=2)
                for s2 in range(2):
                    nc.vector.bn_stats(out=st[:, s2, :], in_=xr[:, s2, :])
                mv = pXA.tile([128, 2], F32, tag="bmv")
                nc.vector.bn_aggr(out=mv, in_=st)
                nc.vector.tensor_copy(out=muA[:, i:i + 1], in_=mv[:, 0:1])
                nc.vector.tensor_copy(out=varA[:, i:i + 1], in_=mv[:, 1:2])
            rstdLN = _rsqrt(nc, pXA, varA, 1.0, 1e-6, [128, 8], "rLN")
            nmr = pXA.tile([128, 8], F32, tag="nmr", bufs=1)
            nc.vector.tensor_tensor(out=nmr, in0=muA, in1=rstdLN, op=AL.mult)
            nc.vector.tensor_scalar(out=nmr, in0=nmr, scalar1=-1.0,
                                    scalar2=None, op0=AL.mult)

            # ------- collect adaln params + broadcast rows ----------
            params_sb = pLN.tile([1, 6 * D], F32, tag="params", bufs=1)
            for r in range(8):
                ag_r = pLN.tile([B, 768], F32, tag="ag1")
                nc.sync.dma_start(out=ag_r, in_=cc1_out[4 * r:4 * (r + 1), :])
                pp1 = pmm("mmf")[:1, :]
                pp2 = pmm("mmi")[:1, 0:256]
                nc.tensor.matmul(pp1, bmask_sb, ag_r[:, 0:512], start=True, stop=True)
                nc.tensor.matmul(pp2, bmask_sb, ag_r[:, 512:768], start=True, stop=True)
                nc.scalar.copy(out=params_sb[:, 768 * r:768 * r + 512], in_=pp1)
                nc.scalar.copy(out=params_sb[:, 768 * r + 512:768 * (r + 1)], in_=pp2)
            nc.vector.tensor_tensor(out=params_sb, in0=params_sb, in1=adb_sb,
                                    op=AL.add)

            def bcast_row(pool, row_ap, bname, plus1=False):
                t = pool.tile([128, D], F32, tag=bname, name=bname)
                for ch in range(0, D, 512):
                    pb = pmm("mm")
                    nc.tensor.matmul(pb, ones_row, row_ap[:, ch:ch + 512],
                                     start=True, stop=True)
                    if plus1:
                        nc.scalar.activation(out=t[:, ch:ch + 512], in_=pb,
                                             func=AF.Identity, bias=1.0)
                    else:
                        nc.scalar.copy(out=t[:, ch:ch + 512], in_=pb)
                return t

            pr = params_sb.rearrange("one (six d) -> one six d", six=6)
            B_sh1 = bcast_row(pb1, pr[:, 0, :], "Bsh1")
            B_sc1 = bcast_row(pb1, pr[:, 1, :], "Bsc1", plus1=True)
            B_g1 = bcast_row(pb1, pr[:, 2, :], "Bg1")
            B_sh2 = bcast_row(pb1, pr[:, 3, :], "Bsh2")
            B_sc2 = bcast_row(pb1, pr[:, 4, :], "Bsc2", plus1=True)
            B_g2 = bcast_row(cst, pr[:, 5, :], "Bg2")
            B_gn = None if GN_ONES else bcast_row(pb1, gnr_sb, "Bgn")
            pLNc.__exit__(None, None, None)

            def quant_stats_sweep(src_get, n, amx, ssx, sb_pool, tagp):
                for i in range(n):
                    s = src_get(i)
                    nc.vector.tensor_reduce(out=amx[:, i:i + 1], in_=s, axis=AX.X,
                                            op=AL.max, apply_absolute_value=True)
                    scr = sb_pool.tile([128, s.free_size()], F32, bufs=1,
                                       tag=tagp + "sq", name=tagp + "sq")
                    nc.scalar.activation(out=scr, in_=s, func=AF.Square,
                                         accum_out=ssx[:, i:i + 1])

            def quant_batch(amx, ssx, dk, q127, dqt, dq_scaled, iws_scaled,
                            sb_pool, tagp):
                ncol = amx.free_size()
                amc = sb_pool.tile([128, ncol], F32, tag=tagp + "amc", name=tagp + "amc")
                nc.vector.tensor_scalar(out=amc, in0=amx, scalar1=1e-5,
                                        scalar2=None, op0=AL.max)
                rec = sb_pool.tile([128, ncol], F32, tag=tagp + "rec", name=tagp + "rec")
                nc.vector.reciprocal(out=rec, in_=amc)
                nc.vector.tensor_scalar(out=q127, in0=rec, scalar1=127.0,
                                        scalar2=None, op0=AL.mult)
                rs = _rsqrt(nc, sb_pool, ssx, 1.0 / dk, 1e-8, [128, ncol], tagp + "rs")
                nc.vector.tensor_tensor(out=dqt, in0=amc, in1=rs, op=AL.mult)
                nc.vector.tensor_scalar(out=dqt, in0=dqt, scalar1=1.0 / 127.0,
                                        scalar2=None, op0=AL.mult)
                if dq_scaled is not None:
                    nc.vector.tensor_scalar(out=dq_scaled, in0=dqt,
                                            scalar1=float(iws_scaled),
                                            scalar2=None, op0=AL.mult)

            cp_state = [0]

            def psum_copy(dst_ap, src_ap):
                k = cp_state[0] % 2
                cp_state[0] += 1
                if k == 0:
                    nc.scalar.copy(out=dst_ap, in_=src_ap)
                else:
                    nc.vector.tensor_copy(out=dst_ap, in_=src_ap)

            def quant_bf(src, q_col, dst, i, sb_pool, tagp):
                """src [128, D] f32 (tokens on partitions) -> int8-grid bf16,
                transposed to dst[:, j, 128i:...] (feature-major)."""
                t2 = sb_pool.tile([128, D], F32, bufs=2, tag=tagp + "t2",
                                  name=tagp + "t2")
                nc.scalar.activation(out=t2, in_=src, func=AF.Identity,
                                     scale=q_col, bias=cC[:, 0:1])
                kq = sb_pool.tile([128, D], BF, bufs=2, tag=tagp + "kq",
                                  name=tagp + "kq")
                nc.vector.tensor_scalar(out=kq, in0=t2, scalar1=C_MAGIC,
                                        scalar2=None, op0=AL.subtract)
                for g4 in range(0, 8, 4):
                    tph = ps.tile([128, 512], BF, tag="tpx", name="tpx", bufs=2)
                    for jj in range(4):
                        jb = g4 + jj
                        nc.tensor.transpose(tph[:, 128 * jj:128 * (jj + 1)],
                                            kq[:, 128 * jb:128 * (jb + 1)], identb)
                    psum_copy(dst[:, g4:g4 + 4, 128 * i:128 * (i + 1)],
                              tph[:].rearrange("p (a q) -> p a q", a=4))

            # ---------------- phase A: LN + modulate + quant ----------------
            pXQc = tc.tile_pool(name="pXQ", bufs=1, side="right")
            pXQ = pXQc.__enter__()
            xqT = pXQ.tile([128, 8, D], BF)
            with tc.tile_pool(name="pa", bufs=2) as pa:
                amA = pa.tile([128, 8], F32, tag="amA")
                ssA = pa.tile([128, 8], F32, tag="ssA")
                moda = pa.tile([128, 8, D], F32, tag="moda", bufs=1)
                for i in range(8):
                    u = pa.tile([128, D], F32, tag="u", bufs=2)
                    nc.scalar.activation(out=u, in_=xa[:, i, :], func=AF.Identity,
                                         scale=rstdLN[:, i:i + 1],
                                         bias=nmr[:, i:i + 1])
                    tt = pa.tile([128, D], F32, tag="tt", bufs=2)
                    nc.vector.tensor_tensor(out=tt, in0=u, in1=B_sc1, op=AL.mult)
                    nc.vector.tensor_tensor(out=moda[:, i, :], in0=tt, in1=B_sh1,
                                            op=AL.add)
                quant_stats_sweep(lambda i: moda[:, i, :], 8, amA, ssA, pa, "qa")
                quant_batch(amA, ssA, D, q127A, dqA, dqAg, iw["g"], pa, "qa")
                for i in range(8):
                    nc.sync.dma_start(
                        out=dqrow_d[128 * i:128 * (i + 1)].rearrange(
                            "(p one) -> p one", one=1),
                        in_=dqA[:, i:i + 1])
                dqrow_sb = pa.tile([1, D], F32, tag="dqrow")
                nc.sync.dma_start(out=dqrow_sb,
                                  in_=dqrow_d[:].rearrange("(one d) -> one d", one=1))
                oi = pa.tile([1, 128], F32, tag="oi")
                nc.vector.memset(oi, float(iw["i"]))
                of = pa.tile([1, 128], F32, tag="of")
                nc.vector.memset(of, float(iw["f"]))
                for ch in range(0, D, 512):
                    pb = pmm("mm")
                    nc.tensor.matmul(pb, oi, dqrow_sb[:, ch:ch + 512],
                                     start=True, stop=True)
                    nc.scalar.copy(out=Sb_i[:, ch:ch + 512], in_=pb)
                    pb2 = pmm("mm")
                    nc.tensor.matmul(pb2, of, dqrow_sb[:, ch:ch + 512],
                                     start=True, stop=True)
                    nc.vector.tensor_copy(out=Sb_f[:, ch:ch + 512], in_=pb2)
                for i in range(8):
                    quant_bf(moda[:, i, :], q127A[:, i:i + 1], xqT, i, pa, "ra")
            pXAc.__exit__(None, None, None)

            # ---------------- phase B: i/f matmuls + scan ----------------
            pSGc = tc.tile_pool(name="pSG", bufs=1)   # gs + hT [B..o-end]
            pSG = pSGc.__enter__()
            gs = pSG.tile([128, 8, D], F32, tag="gs")
            hT = pSG.tile([128, 8, D], F32, tag="hT")
            pHAc = tc.tile_pool(name="pHA", bufs=1)   # ha [scan..fixup]
            pHA = pHAc.__enter__()
            ha = pHA.tile([128, 8, TOK], F32)
            pbsc = tc.tile_pool(name="pbs", bufs=1)   # scan scratch
            pb = pbsc.__enter__()
            for m in range(8):
                ft = pb.tile([128, TOK], F32, tag="ftm")
                it = pb.tile([128, TOK], F32, tag="itm")
                for cki, ck in enumerate(range(0, TOK, 512)):
                    pf = pmm("mmf")
                    pi = pmm("mmi")
                    for j in range(8):
                        nc.tensor.matmul(pf, wf_all[:, j, m, :],
                                         xqT[:, j, ck:ck + 512],
                                         start=(j == 0), stop=(j == 7))
                    for j in range(8):
                        nc.tensor.matmul(pi, wi_all[:, j, m, :],
                                         xqT[:, j, ck:ck + 512],
                                         start=(j == 0), stop=(j == 7))
                    nc.vector.tensor_tensor(out=ft[:, ck:ck + 512], in0=pf,
                                            in1=Sb_f[:, ck:ck + 512], op=AL.mult)
                    nc.vector.tensor_tensor(out=it[:, ck:ck + 512], in0=pi,
                                            in1=Sb_i[:, ck:ck + 512], op=AL.mult)
                sigf = pb.tile([128, TOK], F32, tag="sigf")
                nc.scalar.activation(out=sigf, in_=ft, func=AF.Sigmoid)
                sili = pb.tile([128, TOK], F32, tag="sili")
                nc.scalar.activation(out=sili, in_=it, func=AF.Silu)
                omf = pb.tile([128, TOK], F32, tag="omf")
                nc.vector.tensor_scalar(out=omf, in0=sigf, scalar1=-1.0,
                                        scalar2=1.0, op0=AL.mult, op1=AL.add)
                ifin = pb.tile([128, TOK], F32, tag="ifin")
                nc.vector.tensor_tensor(out=ifin, in0=sili, in1=omf, op=AL.mult)
                nc.vector.tensor_tensor_scan(ha[:, m, :], sigf, ifin, 0.0,
                                             op0=AL.mult, op1=AL.add)
                cam = pb.tile([128, TOK], F32, tag="cam", bufs=1)
                nc.vector.tensor_tensor_scan(cam, sigf, sigf, 1.0,
                                             op0=AL.mult, op1=AL.bypass)
                nc.sync.dma_start(out=ca_d[128 * m:128 * (m + 1), :], in_=cam)
            nc.sync.dma_start(
                out=cc2_in[:].rearrange("(a p) -> p a", p=128),
                in_=ha[:, :, TOK - 1:TOK].rearrange("p a one -> p (a one)"))
            nc.gpsimd.collective_compute(
                "AllGather", AL.bypass, ins=[cc2_in[:]], outs=[cc2_out[:]],
                replica_groups=RG)
            pbsc.__exit__(None, None, None)

            # ---- overlap the collective: g = silu(xq @ wg) * gnorm ----
            pb2c = tc.tile_pool(name="pb2", bufs=1)   # g + fixup scratch
            pb2 = pb2c.__enter__()
            for t in range(8):
                for ck in range(0, D, 512):
                    pg = pmm("mm")
                    for j in range(8):
                        nc.tensor.matmul(pg, xqT[:, j,
                                                 128 * t:128 * (t + 1)],
                                         wg_sb[:, j, ck:ck + 512],
                                         start=(j == 0), stop=(j == 7))
                    if GN_ONES:
                        nc.scalar.activation(out=gs[:, t, ck:ck + 512], in_=pg,
                                             func=AF.Silu, scale=dqAg[:, t:t + 1])
                    else:
                        scr = pb2.tile([128, 512], F32, tag="gscr", bufs=2)
                        nc.scalar.activation(out=scr, in_=pg, func=AF.Silu,
                                             scale=dqAg[:, t:t + 1])
                        nc.vector.tensor_tensor(out=gs[:, t, ck:ck + 512], in0=scr,
                                                in1=B_gn[:, ck:ck + 512], op=AL.mult)
            pXQc.__exit__(None, None, None)
            pW1c.__exit__(None, None, None)

            # ---- carry fixup + transpose h -> hT ----
            ag2 = pb2.tile([N_CORES, D], F32, tag="ag2")
            nc.sync.dma_start(out=ag2, in_=cc2_out[:, :])
            for m in range(8):
                pc0 = pmm("mm")[:, 0:1]
                nc.tensor.matmul(pc0, ag2[:, 128 * m:128 * (m + 1)], mask_sb,
                                 start=True, stop=True)
                carry = pb2.tile([128, 1], F32, tag="carry", bufs=2)
                nc.scalar.copy(out=carry, in_=pc0)
                cam2 = pb2.tile([128, TOK], F32, tag="cam2", bufs=2)
                nc.sync.dma_start(out=cam2, in_=ca_d[128 * m:128 * (m + 1), :])
                hfix = pb2.tile([128, TOK], F32, tag="hfix", bufs=2)
                nc.vector.scalar_tensor_tensor(out=hfix, in0=cam2,
                                               scalar=carry, in1=ha[:, m, :],
                                               op0=AL.mult, op1=AL.add)
                for g4 in range(0, 8, 4):
                    tp = pmm("mm")
                    for jj in range(4):
                        t_i = g4 + jj
                        nc.tensor.transpose(tp[:, 128 * jj:128 * (jj + 1)],
                                            hfix[:, 128 * t_i:128 * (t_i + 1)],
                                            identf)
                    psum_copy(hT[:, g4:g4 + 4, 128 * m:128 * (m + 1)],
                              tp[:].rearrange("p (a q) -> p a q", a=4))
            pb2c.__exit__(None, None, None)
            pHAc.__exit__(None, None, None)

            # ------- o-stage + wo + LN2, pipelined in halves of 4 blocks ----
            pX2c = tc.tile_pool(name="pX2", bufs=1, side="right")
            x2qT = pX2c.__enter__().tile([128, 8, D], BF)
            pOQc = tc.tile_pool(name="pOQ", bufs=1, side="right")
            pOQ = pOQc.__enter__()
            oqT = pOQ.tile([128, 8, D], BF)
            pGWc = tc.tile_pool(name="pGW", bufs=1, side="right")
            pDWc = tc.tile_pool(name="pDW", bufs=1, side="right")
            po1c = tc.tile_pool(name="po1", bufs=2)
            po = po1c.__enter__()
            if True:
                wo_sb = po.tile([128, 8, D], F8, tag="wosb", bufs=1)
                nc.sync.dma_start(out=wo_sb,
                                  in_=woT[:, :].rearrange("(a p) q -> p a q", p=128))
                mshA = po.tile([128, 8, 16], F32, tag="msh", bufs=1)
                amO = po.tile([128, 8], F32, tag="amO", bufs=1)
                ssO = po.tile([128, 8], F32, tag="ssO", bufs=1)
                muC = cst.tile([128, 8], F32, tag="muC")
                varC = cst.tile([128, 8], F32, tag="varC")
                for g in range(2):
                    sl = slice(4 * g, 4 * g + 4)
                    for t in range(4 * g, 4 * g + 4):
                        sq = po.tile([128, D], F32, tag="sq", bufs=2)
                        nc.vector.tensor_tensor(out=sq, in0=hT[:, t, :],
                                                in1=hT[:, t, :], op=AL.mult)
                        nc.vector.tensor_reduce(
                            out=mshA[:, t, :],
                            in_=sq.rearrange("p (h d) -> p h d", h=NH),
                            axis=AX.X, op=AL.add)
                    rstdH = _rsqrt(nc, po,
                                   mshA[:, sl, :].rearrange("p a b -> p (a b)"),
                                   1.0 / HD, 1e-5, [128, 64], "rH")
                    rH = rstdH.rearrange("p (a b) -> p a b", a=4)
                    for idx, t in enumerate(range(4 * g, 4 * g + 4)):
                        hn = po.tile([128, D], F32, tag="hn", bufs=2)
                        rb = bass.AP(tensor=rH.tensor, offset=rH[:, idx, :].offset,
                                     ap=[rH.ap[0], [1, NH], [0, HD]])
                        nc.vector.tensor_tensor(
                            out=hn.rearrange("p (h d) -> p h d", h=NH),
                            in0=hT[:, t, :].rearrange("p (h d) -> p h d", h=NH),
                            in1=rb, op=AL.mult)
                        # oa overwrites gs in place
                        nc.vector.tensor_tensor(out=gs[:, t, :], in0=hn,
                                                in1=gs[:, t, :], op=AL.mult)
                        nc.vector.tensor_reduce(out=amO[:, t:t + 1],
                                                in_=gs[:, t, :], axis=AX.X,
                                                op=AL.max,
                                                apply_absolute_value=True)
                        scr = po.tile([128, D], F32, bufs=1, tag="qosq",
                                      name="qosq")
                        nc.scalar.activation(out=scr, in_=gs[:, t, :],
                                             func=AF.Square,
                                             accum_out=ssO[:, t:t + 1])
                    quant_batch(amO[:, sl], ssO[:, sl], D, q127O[:, sl],
                                dqOo[:, sl], dqOo[:, sl], iw["o"], po, "qo")
                    for t in range(4 * g, 4 * g + 4):
                        quant_bf(gs[:, t, :], q127O[:, t:t + 1], oqT, t, po, "ro")
                        xa2 = po.tile([128, D], F32, tag="xa2", bufs=2)
                        nc.sync.dma_start(out=xa2,
                                          in_=x_sl[128 * t:128 * (t + 1), :])
                        xn = po.tile([128, D], F32, tag="xn", bufs=2)
                        for ck in range(0, D, 512):
                            pw = pmm("mmf")
                            for j in range(8):
                                nc.tensor.matmul(pw, oqT[:, j,
                                                         128 * t:128 * (t + 1)],
                                                 wo_sb[:, j, ck:ck + 512],
                                                 start=(j == 0), stop=(j == 7))
                            at = po.tile([128, 512], F32, tag="at", bufs=2)
                            nc.vector.tensor_scalar(out=at, in0=pw,
                                                    scalar1=dqOo[:, t:t + 1],
                                                    scalar2=None, op0=AL.mult)
                            ug = po.tile([128, 512], F32, tag="ug", bufs=2)
                            nc.vector.tensor_tensor(out=ug, in0=at,
                                                    in1=B_g1[:, ck:ck + 512],
                                                    op=AL.mult)
                            nc.vector.tensor_tensor(out=xn[:, ck:ck + 512],
                                                    in0=ug,
                                                    in1=xa2[:, ck:ck + 512],
                                                    op=AL.add)
                        nc.sync.dma_start(out=xnew_d[128 * t:128 * (t + 1), :],
                                          in_=xn)
                        st = po.tile([128, 2, 6], F32, tag="bst2")
                        xrr = xn.rearrange("p (s d) -> p s d", s=2)
                        for s2 in range(2):
                            nc.vector.bn_stats(out=st[:, s2, :], in_=xrr[:, s2, :])
                        mv = po.tile([128, 2], F32, tag="bmv2")
                        nc.vector.bn_aggr(out=mv, in_=st)
                        nc.vector.tensor_copy(out=muC[:, t:t + 1], in_=mv[:, 0:1])
                        nc.vector.tensor_copy(out=varC[:, t:t + 1], in_=mv[:, 1:2])
                pOQc.__exit__(None, None, None)
            po1c.__exit__(None, None, None)
            pSGc.__exit__(None, None, None)
            gw_sb = pGWc.__enter__().tile([128, 8, 2 * MLP], F8)
            nc.sync.dma_start(
                out=gw_sb,
                in_=gwT[:, :].rearrange("(a p) q -> p a q", p=128))
            po2c = tc.tile_pool(name="po2", bufs=2)
            po = po2c.__enter__()
            if True:
                rstdC = _rsqrt(nc, po, varC, 1.0, 1e-6, [128, 8], "rC")
                nmrC = po.tile([128, 8], F32, tag="nmrC", bufs=1)
                nc.vector.tensor_tensor(out=nmrC, in0=muC, in1=rstdC, op=AL.mult)
                nc.vector.tensor_scalar(out=nmrC, in0=nmrC, scalar1=-1.0,
                                        scalar2=None, op0=AL.mult)
                amC = po.tile([128, 8], F32, tag="amC", bufs=1)
                ssC = po.tile([128, 8], F32, tag="ssC", bufs=1)
                for g in range(2):
                    sl = slice(4 * g, 4 * g + 4)
                    m2s = []
                    for t in range(4 * g, 4 * g + 4):
                        xn2 = po.tile([128, D], F32, tag="xn2", bufs=2)
                        nc.sync.dma_start(out=xn2,
                                          in_=xnew_d[128 * t:128 * (t + 1), :])
                        u2 = po.tile([128, D], F32, tag="u2", bufs=2)
                        nc.scalar.activation(out=u2, in_=xn2, func=AF.Identity,
                                             scale=rstdC[:, t:t + 1],
                                             bias=nmrC[:, t:t + 1])
                        tt2 = po.tile([128, D], F32, tag="tt2", bufs=2)
                        nc.vector.tensor_tensor(out=tt2, in0=u2, in1=B_sc2,
                                                op=AL.mult)
                        m2 = po.tile([128, D], F32, tag="m2", bufs=4)
                        nc.vector.tensor_tensor(out=m2, in0=tt2, in1=B_sh2,
                                                op=AL.add)
                        m2s.append(m2)
                        nc.vector.tensor_reduce(out=amC[:, t:t + 1], in_=m2,
                                                axis=AX.X, op=AL.max,
                                                apply_absolute_value=True)
                        scr = po.tile([128, D], F32, bufs=1, tag="qcsq",
                                      name="qcsq")
                        nc.scalar.activation(out=scr, in_=m2, func=AF.Square,
                                             accum_out=ssC[:, t:t + 1])
                    quant_batch(amC[:, sl], ssC[:, sl], D, q127C[:, sl],
                                dqCg[:, sl], dqCg[:, sl], iw["gate"], po, "qc")
                    for idx, t in enumerate(range(4 * g, 4 * g + 4)):
                        quant_bf(m2s[idx], q127C[:, t:t + 1], x2qT, t, po, "rc")
            po2c.__exit__(None, None, None)
            pB1c.__exit__(None, None, None)

            # ---------------- phase D: MLP (SBUF-resident, pipelined) ------
            dw_sb = pDWc.__enter__().tile([128, 32, D], F8)
            nc.sync.dma_start(out=dw_sb,
                              in_=dwT[:, :].rearrange("(a p) q -> p a q", p=128))
            with tc.tile_pool(name="pd", bufs=2) as pd:
                def emit_gate_half(t, h2_h, amD_t, ssD_t, half):
                    for ck in range(4):
                        c0 = 2048 * half + 512 * ck
                        pg = pmm("mmf")
                        py = pmm("mmi")
                        for j in range(8):
                            nc.tensor.matmul(pg, x2qT[:, j,
                                                      128 * t:128 * (t + 1)],
                                             gw_sb[:, j, c0:c0 + 512],
                                             start=(j == 0), stop=(j == 7))
                        for j in range(8):
                            nc.tensor.matmul(py, x2qT[:, j,
                                                      128 * t:128 * (t + 1)],
                                             gw_sb[:, j,
                                                   MLP + c0:MLP + c0 + 512],
                                             start=(j == 0), stop=(j == 7))
                        sil = pd.tile([128, 512], F32, tag="sil", bufs=1)
                        nc.scalar.activation(out=sil, in_=pg, func=AF.Silu,
                                             scale=dqCg[:, t:t + 1])
                        hc = 512 * ck
                        nc.vector.scalar_tensor_tensor(
                            out=h2_h[:, hc:hc + 512], in0=py,
                            scalar=dqCg[:, t:t + 1], in1=sil,
                            op0=AL.mult, op1=AL.mult)
                        cki = 4 * half + ck
                        nc.vector.tensor_reduce(out=amD_t[:, cki:cki + 1],
                                                in_=h2_h[:, hc:hc + 512],
                                                axis=AX.X, op=AL.max,
                                                apply_absolute_value=True)
                        scr = pd.tile([128, 512], F32, tag="sqd", bufs=1)
                        nc.scalar.activation(out=scr, in_=h2_h[:, hc:hc + 512],
                                             func=AF.Square,
                                             accum_out=ssD_t[:, cki:cki + 1])

                def emit_quant_h2(t, h2_hs, amD_t, ssD_t, h2q_t):
                    amD = pd.tile([128, 1], F32, tag="amD", bufs=2)
                    nc.vector.tensor_reduce(out=amD, in_=amD_t, axis=AX.X,
                                            op=AL.max)
                    ssD = pd.tile([128, 1], F32, tag="ssD", bufs=2)
                    nc.vector.tensor_reduce(out=ssD, in_=ssD_t, axis=AX.X,
                                            op=AL.add)
                    amc = pd.tile([128, 1], F32, tag="qdamc", bufs=2)
                    nc.vector.tensor_scalar(out=amc, in0=amD, scalar1=1e-5,
                                            scalar2=None, op0=AL.max)
                    rec = pd.tile([128, 1], F32, tag="qdrec", bufs=2)
                    nc.vector.reciprocal(out=rec, in_=amc)
                    q127 = pd.tile([128, 1], F32, tag="qdq", bufs=2)
                    nc.vector.tensor_scalar(out=q127, in0=rec, scalar1=127.0,
                                            scalar2=None, op0=AL.mult)
                    rs = _rsqrt(nc, pd, ssD, 1.0 / MLP, 1e-8, [128, 1], "rD")
                    dq = pd.tile([128, 1], F32, tag="qddq", bufs=2)
                    nc.vector.tensor_tensor(out=dq, in0=amc, in1=rs, op=AL.mult)
                    nc.vector.tensor_scalar(out=dq, in0=dq,
                                            scalar1=float(iw["down"]) / 127.0,
                                            scalar2=None, op0=AL.mult)
                    for half in range(2):
                        h2_h = h2_hs[half]
                        t2d = pd.tile([128, 2048], F32, tag="t2d", bufs=1)
                        nc.scalar.activation(out=t2d, in_=h2_h, func=AF.Identity,
                                             scale=q127[:, 0:1], bias=cC[:, 0:1])
                        kq = pd.tile([128, 2048], BF, tag="kq", bufs=1)
                        nc.vector.tensor_scalar(out=kq, in0=t2d, scalar1=C_MAGIC,
                                                scalar2=None, op0=AL.subtract)
                        for g8 in range(0, 16, 4):
                            tp = ps.tile([128, 512], BF, tag="tpx", name="tpx",
                                         bufs=2)
                            for jj in range(4):
                                j2 = g8 + jj
                                nc.tensor.transpose(
                                    tp[:, 128 * jj:128 * (jj + 1)],
                                    kq[:, 128 * j2:128 * (j2 + 1)], identb)
                            psum_copy(h2q_t[:, 16 * half + g8:16 * half + g8 + 4, :],
                                      tp[:].rearrange("p (a q) -> p a q", a=4))
                    return dq

                def emit_down(t, h2q_t, dq):
                    xn3 = pd.tile([128, D], F32, tag="xn3", bufs=1)
                    nc.sync.dma_start(out=xn3,
                                      in_=xnew_d[128 * t:128 * (t + 1), :])
                    outt = pd.tile([128, D], F32, tag="outt", bufs=2)
                    for ck in range(0, D, 512):
                        pdn = pmm("mm")
                        for j2 in range(32):
                            nc.tensor.matmul(pdn, h2q_t[:, j2, :],
                                             dw_sb[:, j2, ck:ck + 512],
                                             start=(j2 == 0), stop=(j2 == 31))
                        u2 = pd.tile([128, 512], F32, tag="u2d", bufs=2)
                        nc.vector.tensor_scalar(out=u2, in0=pdn, scalar1=dq,
                                                scalar2=None, op0=AL.mult)
                        v2 = pd.tile([128, 512], F32, tag="v2d", bufs=2)
                        nc.vector.tensor_tensor(out=v2, in0=u2,
                                                in1=B_g2[:, ck:ck + 512], op=AL.mult)
                        nc.vector.tensor_tensor(out=outt[:, ck:ck + 512], in0=v2,
                                                in1=xn3[:, ck:ck + 512], op=AL.add)
                    nc.sync.dma_start(out=out_sl[128 * t:128 * (t + 1), :], in_=outt)

                prev = None
                for t in range(8):
                    h2_a = pd.tile([128, 2048], F32, tag="h2h", bufs=4)
                    h2_b = pd.tile([128, 2048], F32, tag="h2h", bufs=4)
                    amD_t = pd.tile([128, 8], F32, tag="amDt", bufs=2)
                    ssD_t = pd.tile([128, 8], F32, tag="ssDt", bufs=2)
                    emit_gate_half(t, h2_a, amD_t, ssD_t, 0)
                    emit_gate_half(t, h2_b, amD_t, ssD_t, 1)
                    if prev is not None:
                        pt, ph2s, pam, pss = prev
                        h2q_t = pd.tile([128, 32, 128], BF, tag="h2qt", bufs=2)
                        dq = emit_quant_h2(pt, ph2s, pam, pss, h2q_t)
                        emit_down(pt, h2q_t, dq)
                    prev = (t, (h2_a, h2_b), amD_t, ssD_t)
                pt, ph2s, pam, pss = prev
                h2q_t = pd.tile([128, 32, 128], BF, tag="h2qt", bufs=2)
                dq = emit_quant_h2(pt, ph2s, pam, pss, h2q_t)
                emit_down(pt, h2q_t, dq)
            pDWc.__exit__(None, None, None)
            pGWc.__exit__(None, None, None)
            pX2c.__exit__(None, None, None)

    nc.finalize()
    return nc


@functools.lru_cache(maxsize=2)
def _build_cached(iw_items, gn_ones):
    return _build(dict(iw_items), gn_ones)


def kernel(x, c, adaln_w, adaln_b, wi, wf, wg, gnorm_w, wo, gate_w, down_w):
    x = np.ascontiguousarray(np.asarray(x, dtype=np.float32))
    c = np.ascontiguousarray(np.asarray(c, dtype=np.float32))
    adaln_w = np.asarray(adaln_w, dtype=np.float32)
    adaln_b = np.asarray(adaln_b, dtype=np.float32)
    gnorm_w = np.asarray(gnorm_w, dtype=np.float32)

    mi, iwi = _quant_w(np.asarray(wi, dtype=np.float32))
    mf, iwf = _quant_w(np.asarray(wf, dtype=np.float32))
    mg, iwg = _quant_w(np.asarray(wg, dtype=np.float32))
    mo, iwo = _quant_w(np.asarray(wo, dtype=np.float32))
    mgate, iwgate = _quant_w(np.asarray(gate_w, dtype=np.float32))
    mdown, iwdown = _quant_w(np.asarray(down_w, dtype=np.float32))

    iw = {"i": float(iwi), "f": float(iwf), "g": float(iwg), "o": float(iwo),
          "gate": float(iwgate), "down": float(iwdown)}
    gn_ones = bool(np.allclose(gnorm_w, 1.0))
    nc = _build_cached(tuple(sorted(iw.items())), gn_ones)

    wiT_h = np.ascontiguousarray(mi.T)
    wfT_h = np.ascontiguousarray(mf.T)
    wgT_h = np.ascontiguousarray(mg.T)
    woT_h = np.ascontiguousarray(mo.T)
    gwT_h = np.ascontiguousarray(mgate.T)
    dwT_h = np.ascontiguousarray(mdown.T)
    adwT = np.ascontiguousarray(adaln_w.T)          # [D, 6D]
    adb_row_h = np.ascontiguousarray(adaln_b[None, :])
    gnr_h = np.ascontiguousarray(np.tile(gnorm_w, NH)[None, :])
    c_cols_h = np.ascontiguousarray(
        c.T.reshape(8, 128, B).transpose(1, 0, 2))   # [128, 8, B]

    in_maps = []
    for core in range(N_CORES):
        b, half = core // 2, core % 2
        mask = np.zeros((N_CORES, 1), np.float32)
        if half == 1:
            mask[core - 1, 0] = 1.0
        bm = np.zeros((B, 1), np.float32)
        bm[b, 0] = 1.0
        in_maps.append({
            "x_sl": np.ascontiguousarray(x[b, half * TOK:(half + 1) * TOK, :]),
            "c_cols": c_cols_h,
            "adw_sl": np.ascontiguousarray(adwT[:, 768 * core:768 * (core + 1)]),
            "adb_row": adb_row_h,
            "mask8": mask,
            "bmask": bm,
            "gnr": gnr_h,
            "wiT": wiT_h, "wfT": wfT_h, "wgT": wgT_h, "woT": woT_h,
            "gwT": gwT_h, "dwT": dwT_h,
        })

    res = run_bass_kernel_spmd(nc, in_maps, core_ids=list(range(N_CORES)))
    out = np.zeros((B, T, D), np.float32)
    for core in range(N_CORES):
        b, half = core // 2, core % 2
        out[b, half * TOK:(half + 1) * TOK, :] = res.results[core]["out_sl"]
    return out
